# revision 25
# baseline (speedup 1.0000x reference)
"""Trainium2 Bass kernel for a pre-norm transformer block (E=512, H=2048, NH=8, N=4096).

Sharding: sequence-parallel over 8 NeuronCores. Each core computes the full K/V
projection (needs all 4096 tokens) but only its own 512-token slice of queries,
attention output, MLP and residuals. No collectives; host concatenates slices.

v2: every large matmul runs in fp8(e4m3) DoubleRow perf mode (2 contraction
rows per PE cell, 0.5 cycles/row = 4x fewer PE cycles than bf16 at these
shapes). Contraction pairs are realized with host-side weight reshapes
([K, out] -> [K/256, 128, 2, out]); Q/K additionally get a head-dim pair
permutation (d -> (d//2, d%2)) so the 64-wide scores contraction becomes a
[32, 2, *] DoubleRow operand. Softmax exp is split across two engines by
query column: ACT runs native Exp -> fp8e5m2, DVE runs the Schraudolph exp2
bit trick (uint8 = round(0.72135*score + 60) bitcast to e5m2). Each softmax
row (head, q) is served by exactly one engine, so each row's uniform rounding
bias cancels in the softmax normalization (the denominator rides the PV
matmul as a 65th ones-column of V). The V bias is folded into the proj bias
on the host (exact algebra). The residual stream stays fp32.
"""
import sys

sys.path.insert(0, "/opt/trn_rl_repo")
sys.path.insert(0, "/opt/pypackages")

import numpy as np

E, H, NH, HD = 512, 2048, 8, 64
T, NCORES = 4096, 8
TC = T // NCORES          # tokens per core
P = 128
ET = E // P               # 4  feature tiles of E
HT = H // P               # 16 feature tiles of H
KT = T // P               # 32 key-token tiles
EPS = 1e-5
QA = 300                  # q columns per exp tile handled by ACT (rest: DVE)
EXP_A = 0.125 * 4 * np.log2(np.e)   # DVE bit-trick slope (e5m2, scores pre-scaled)
EXP_B = 60.0                        # e5m2 exponent-bias offset (4*15)

_BUILT = None


def _build():
    import concourse.bacc as bacc
    import concourse.mybir as mybir
    import concourse.tile as tile

    dt = mybir.dt
    F32 = dt.float32
    BF16 = dt.bfloat16
    F8 = dt.float8e4

    nc = bacc.Bacc("TRN2", target_bir_lowering=False, debug=False, num_devices=NCORES)

    d = {}
    d["d_xT"] = nc.dram_tensor("xT", [E, T], BF16, kind="ExternalInput").ap()
    d["d_xsT"] = nc.dram_tensor("xsT", [E, TC], F32, kind="ExternalInput").ap()
    # DoubleRow stationary layouts: [n_pair_groups, 128, 2, out_cols]
    d["d_wkv"] = nc.dram_tensor("wkv8", [2, P, 2, 2 * E], F8, kind="ExternalInput").ap()
    d["d_wq"] = nc.dram_tensor("wq8", [2, P, 2, E], F8, kind="ExternalInput").ap()
    d["d_wproj"] = nc.dram_tensor("wproj8", [2, P, 2, E], F8, kind="ExternalInput").ap()
    d["d_wfc1T"] = nc.dram_tensor("wfc1T", [E, H], BF16, kind="ExternalInput").ap()
    d["d_wfc2T"] = nc.dram_tensor("wfc2T", [H, H], BF16, kind="ExternalInput").ap()
    d["d_wfc3T"] = nc.dram_tensor("wfc3T", [H, E], BF16, kind="ExternalInput").ap()
    d["d_bqp"] = nc.dram_tensor("bqp", [P, ET], F32, kind="ExternalInput").ap()
    d["d_bkp"] = nc.dram_tensor("bkp", [P, ET], F32, kind="ExternalInput").ap()
    d["d_bproj"] = nc.dram_tensor("bproj", [E], F32, kind="ExternalInput").ap()
    d["d_bfc1"] = nc.dram_tensor("bfc1", [H], F32, kind="ExternalInput").ap()
    d["d_bfc2"] = nc.dram_tensor("bfc2", [H], F32, kind="ExternalInput").ap()
    d["d_bfc3"] = nc.dram_tensor("bfc3", [E], F32, kind="ExternalInput").ap()
    d["d_lng"] = nc.dram_tensor("lng", [E], F32, kind="ExternalInput").ap()
    d["d_lnb"] = nc.dram_tensor("lnb", [E], F32, kind="ExternalInput").ap()
    d["d_outT"] = nc.dram_tensor("outT", [E, TC], F32, kind="ExternalOutput").ap()

    with tile.TileContext(nc) as tc:
        _emit(nc, tc, tile, mybir, d)

    nc.compile()
    return nc


def _emit(nc, tc, tile, mybir, d):
    dt = mybir.dt
    AF = mybir.ActivationFunctionType
    OP = mybir.AluOpType
    PM = mybir.MatmulPerfMode
    F32 = dt.float32
    BF16 = dt.bfloat16
    F8 = dt.float8e4
    F8E5 = dt.float8e5
    U8 = dt.uint8

    def pool(**kw):
        p = tc.tile_pool(**kw)
        return p.__enter__(), p

    def close(*ps):
        for p in ps:
            p.__exit__(None, None, None)

    # ---- long-lived pools ----
    consts, _c0 = pool(name="consts", bufs=1, side="left")
    stats, _c1 = pool(name="stats", bufs=3, side="left")
    bcast, _c2 = pool(name="bcast", bufs=3, side="left")
    scratch, _c3 = pool(name="scratch", bufs=6, side="left")
    STAT = [pool(name="ps_stat", bufs=2, space="PSUM")]
    drp, _cd = pool(name="drscratch", bufs=4, space="DRAM")

    # ---- constants ----
    onesf = consts.tile([P, 1], F32)
    nc.vector.memset(onesf[:], 1.0)
    ones_w = consts.tile([P, 1], BF16)
    nc.vector.tensor_copy(ones_w[:], onesf[:])
    eps_t = consts.tile([1, 1], F32)
    nc.vector.memset(eps_t[:], EPS)
    eps_p = consts.tile([P, 1], F32)
    nc.vector.memset(eps_p[:], EPS)

    def ld_vec(dram, n, name):  # [n] f32 -> [P, n//P] per-partition layout
        t = consts.tile([P, n // P], F32, name=name)
        nc.sync.dma_start(t[:], dram.rearrange("(m p) -> p m", p=P))
        return t

    g_sb = ld_vec(d["d_lng"], E, "g_sb")
    b_sb = ld_vec(d["d_lnb"], E, "b_sb")
    bq_sb = consts.tile([P, ET], F32, name="bq_sb")
    nc.sync.dma_start(bq_sb[:], d["d_bqp"])
    bk_sb = consts.tile([P, ET], F32, name="bk_sb")
    nc.sync.dma_start(bk_sb[:], d["d_bkp"])
    bproj_sb = ld_vec(d["d_bproj"], E, "bproj_sb")
    bfc1_sb = ld_vec(d["d_bfc1"], H, "bfc1_sb")
    bfc2_sb = ld_vec(d["d_bfc2"], H, "bfc2_sb")
    bfc3_sb = ld_vec(d["d_bfc3"], E, "bfc3_sb")

    def recip_bcast(dst_bb, src_1w, w, rsqrt=False):
        """dst_bb[P, w] = broadcast(1/src) or broadcast(1/sqrt(src + eps)).

        Spread the w values over 128 partitions via a DRAM bounce, invert
        there, broadcast back with a stride-0 DMA read.
        """
        dr1 = drp.tile([w], F32, tag="dr1", name="dr1")
        nc.sync.dma_start(dr1[None, :], src_1w)
        pk = scratch.tile([P, w // P], F32, tag="rpk", name="rpk")
        nc.sync.dma_start(pk[:], dr1.rearrange("(p f) -> p f", p=P))
        if rsqrt:
            nc.scalar.activation(pk[:], pk[:], AF.Sqrt, bias=eps_p[:])
        nc.vector.reciprocal(pk[:], pk[:])
        dr2 = drp.tile([w], F32, tag="dr2", name="dr2")
        nc.sync.dma_start(dr2.rearrange("(p f) -> p f", p=P), pk[:])
        src_b = dr2[None, :].to_broadcast((P, w))
        if dst_bb.dtype == F32:
            nc.sync.dma_start(dst_bb, src_b)
        else:
            nc.gpsimd.dma_start(dst_bb, src_b)

    def bcast_dram(dst_bb, src_1w, w):
        dr = drp.tile([w], F32, tag="drb", name="drb")
        nc.sync.dma_start(dr[None, :], src_1w)
        src_b = dr[None, :].to_broadcast((P, w))
        if dst_bb.dtype == F32:
            nc.sync.dma_start(dst_bb, src_b)
        else:
            nc.gpsimd.dma_start(dst_bb, src_b)

    def ln_stats(src, w, fast=False):
        """LN statistics from bf16 src tiles [P, ET, w]: (mu_b, rs_b) [P, w] bf16."""
        x2s = [scratch.tile([P, w], BF16, tag="ln_x2", name="ln_x2")
               for _ in range(ET)]
        for e in range(ET):
            nc.vector.tensor_mul(x2s[e][:], src[:, e, :], src[:, e, :])
        mu_ps = STAT[0][0].tile([1, w], F32, tag="mu")
        sq_ps = STAT[0][0].tile([1, w], F32, tag="sq")
        for e in range(ET):
            nc.tensor.matmul(mu_ps[:], ones_w[:], src[:, e, :],
                             start=(e == 0), stop=(e == ET - 1))
        for e in range(ET):
            nc.tensor.matmul(sq_ps[:], ones_w[:], x2s[e][:],
                             start=(e == 0), stop=(e == ET - 1))
        return ln_chain(mu_ps, sq_ps, w, fast=fast)

    def ln_chain(mu_ps, sq_ps, w, fast=False):
        mu = stats.tile([1, w], F32, tag="mu_sb", name="mu_sb")
        ms = stats.tile([1, w], F32, tag="ms_sb", name="ms_sb")
        if fast:
            nc.scalar.mul(mu[:], mu_ps[:], 1.0 / E)
            nc.scalar.mul(ms[:], sq_ps[:], 1.0 / E)
        else:
            nc.vector.tensor_scalar_mul(mu[:], mu_ps[:], 1.0 / E)
            nc.vector.tensor_scalar_mul(ms[:], sq_ps[:], 1.0 / E)
        var = stats.tile([1, w], F32, tag="var", name="var")
        nc.vector.tensor_mul(var[:], mu[:], mu[:])
        nc.vector.tensor_sub(var[:], ms[:], var[:])
        mu_b = bcast.tile([P, w], BF16, tag="mu_b", name="mu_b")
        rs_b = bcast.tile([P, w], BF16, tag="rs_b", name="rs_b")
        if fast:
            rstd = consts.tile([1, w], F32, tag="rstd_f", name="rstd_f")
            nc.scalar.activation(rstd[:], var[:], AF.Sqrt, bias=eps_t[:])
            nc.vector.reciprocal(rstd[:], rstd[:])
            mu_w = consts.tile([1, w], BF16, tag="mu_w", name="mu_w")
            rs_w = consts.tile([1, w], BF16, tag="rs_w", name="rs_w")
            nc.vector.tensor_copy(mu_w[:], mu[:])
            nc.vector.tensor_copy(rs_w[:], rstd[:])
            nc.gpsimd.partition_broadcast(mu_b[:], mu_w[:])
            nc.gpsimd.partition_broadcast(rs_b[:], rs_w[:])
        else:
            bcast_dram(mu_b[:], mu[:], w)
            recip_bcast(rs_b[:], var[:], w, rsqrt=True)
        return mu_b, rs_b

    def ln_apply(dst8, src, mu_b, rs_b, w):
        """dst8[P, ET, w] fp8 = LN(src bf16). Last op converts; split ACT/DVE."""
        for e in range(ET):
            t1 = scratch.tile([P, w], BF16, tag="ln_t1", name="ln_t1")
            nc.vector.tensor_sub(t1[:], src[:, e, :], mu_b[:])
            nc.vector.scalar_tensor_tensor(t1[:], t1[:],
                                           g_sb[:, e:e + 1], rs_b[:],
                                           op0=OP.mult, op1=OP.mult)
            if e % 2 == 0:
                nc.scalar.activation(dst8[:, e, :], t1[:], AF.Identity,
                                     bias=b_sb[:, e:e + 1])
            else:
                nc.vector.tensor_scalar_add(dst8[:, e, :], t1[:],
                                            scalar1=b_sb[:, e:e + 1])

    # ---- K/V/Q tensors live until end of attention ----
    # K2/Q2 pair layout for DoubleRow scores: head h = 2u+a lives at
    # partitions [32a, 32a+32) of group u; (p, j) -> head dim d = 2p+j.
    big, h_big = pool(name="big", bufs=1, side="right")
    K2 = big.tile([HD, ET, 2, T], F8)           # [part (2 heads), u, j, tok]
    # V + ones column, padded to 80 so dual-fp8 LDWEIGHTS strides are
    # 16B-aligned (per-head 80, per-k-tile 640)
    V65 = big.tile([P, KT, NH, 80], F8)
    Q2 = big.tile([HD, ET, 2, TC], F8)

    # ones column of V65 (denominator rides the PV matmul)
    nc.vector.tensor_copy(
        V65[:, :, :, HD:HD + 1],
        onesf[:, :, None, None].to_broadcast((P, KT, NH, 1)))

    # ====== phase 1: stream x in 512-token chunks, LN, K/V proj ======
    ps_mm4, h_ps_mm4 = pool(name="ps_mm4", bufs=4, space="PSUM")
    CW = 512
    NCH1 = T // CW
    wqp, h_wqp = pool(name="wq", bufs=1, side="right")
    wq = wqp.tile([P, 2, 2, E], F8)             # [part, u_c, j_c, cols]
    wkvp, h_wkvp = pool(name="wkv", bufs=1, side="right")
    wkv = wkvp.tile([P, 2, 2, 2 * E], F8)       # cols: K-perm 512 | V 512
    xcp, h_xcp = pool(name="xc", bufs=3, side="right")
    hcp, h_hcp = pool(name="hc", bufs=3, side="right")

    def warmup(n, rhs, pool_=None):
        wps = (pool_ or ps_mm4).tile([1, rhs.shape[-1]], F32, tag="mm", name="wps")
        for i in range(n):
            nc.tensor.matmul(wps[:], ones_w[:], rhs,
                             start=(i == 0), stop=(i == n - 1),
                             skip_group_check=True)

    kqst, h_kqst = pool(name="kqst", bufs=6, side="right")

    def shuffle_pairs(dst, stg_t, m, cols):
        """Partition-shuffle staging [128, w] fp8 into the pair layout.

        Staging partition pi = 64a + 32j + p holds head (2m+a), d = 2p+j;
        dest is dst[32a:32a+32, m, j, cols]."""
        for a in range(2):
            for j in range(2):
                nc.sync.dma_start(dst[32 * a:32 * a + 32, m, j, cols],
                                  stg_t[64 * a + 32 * j:64 * a + 32 * j + 32, :])

    def kv_project(hc, ch):
        # K: 4 dense m-tiles; stationary = host-permuted wkv K cols
        for m in range(ET):
            kps = ps_mm4.tile([P, CW], F32, tag="mm", name="kps")
            for u in range(2):
                nc.tensor.matmul(kps[:], wkv[:, u, :, m * P:(m + 1) * P],
                                 hc[:, 2 * u:2 * u + 2, :],
                                 start=(u == 0), stop=(u == 1),
                                 perf_mode=PM.DoubleRow)
            kst = kqst.tile([P, CW], F8, tag="kst", name="kst")
            nc.scalar.activation(kst[:], kps[:], AF.Identity,
                                 bias=bk_sb[:, m:m + 1])
            shuffle_pairs(K2, kst, m, slice(ch * CW, (ch + 1) * CW))
        # V: token-stationary, moving = wkv V cols -> [128 tok, 512 E]
        for t4 in range(CW // P):
            vps = ps_mm4.tile([P, E], F32, tag="mm", name="vps")
            for u in range(2):
                nc.tensor.matmul(vps[:], hc[:, 2 * u:2 * u + 2, t4 * P:(t4 + 1) * P],
                                 wkv[:, u, :, E:2 * E],
                                 start=(u == 0), stop=(u == 1),
                                 perf_mode=PM.DoubleRow)
            kt = ch * (CW // P) + t4
            nc.scalar.activation(
                V65[:, kt, :, 0:HD],
                vps[:].rearrange("p (h d) -> p h d", h=NH), AF.Copy)

    # Q projection resources (emitted mid-pipeline so it overlaps the K/V pass)
    persistA, h_persistA = pool(name="persistA", bufs=1, side="left")
    xs_sb = persistA.tile([P, ET, TC], F32)
    UTs = persistA.tile([P, ET, TC], F8)        # attention out (fp8, feature-major)
    persistQ, h_persistQ = pool(name="persistQ", bufs=1, side="left")
    xs_bf = persistQ.tile([P, ET, TC], BF16)
    hq = persistQ.tile([P, ET, TC], F8)

    def emit_phase2():
        nc.sync.dma_start(xs_sb[:], d["d_xsT"].rearrange("(m p) t -> p m t", p=P))
        for e in range(ET):
            nc.vector.tensor_copy(xs_bf[:, e, :], xs_sb[:, e, :])
        mu_b, rs_b = ln_chain_src(xs_bf, fast=True)
        ln_apply(hq, xs_bf, mu_b, rs_b, TC)
        for m in range(ET):
            qps = ps_mm4.tile([P, TC], F32, tag="mm", name="qps")
            for u in range(2):
                nc.tensor.matmul(qps[:], wq[:, u, :, m * P:(m + 1) * P],
                                 hq[:, 2 * u:2 * u + 2, :],
                                 start=(u == 0), stop=(u == 1),
                                 perf_mode=PM.DoubleRow)
            qst = kqst.tile([P, TC], F8, tag="kst", name="qst")
            nc.scalar.activation(qst[:], qps[:], AF.Identity,
                                 bias=bq_sb[:, m:m + 1])
            shuffle_pairs(Q2, qst, m, slice(0, TC))

    def ln_chain_src(src_bf, fast):
        return ln_stats(src_bf, TC, fast=fast)

    pending = []
    for ch in range(NCH1):
        xc = xcp.tile([P, ET, CW], BF16, tag="xc", name="xc")
        nc.sync.dma_start(
            xc[:],
            d["d_xT"][:, ch * CW:(ch + 1) * CW].rearrange("(m p) t -> p m t", p=P))
        if ch == 0:
            warmup(24, xc[:, 0, :])          # bridge the LN latency at t=0
            nc.sync.dma_start(wkv[:], d["d_wkv"].rearrange("u p j o -> p u j o"))
            nc.sync.dma_start(wq[:], d["d_wq"].rearrange("u p j o -> p u j o"))
        mu_b, rs_b = ln_stats(xc, CW)
        hc = hcp.tile([P, ET, CW], F8, tag="hc", name="hc")
        ln_apply(hc, xc, mu_b, rs_b, CW)
        pending.append((hc, ch))
        if len(pending) == 2:
            kv_project(*pending.pop(0))
    while pending:
        kv_project(*pending.pop(0))
    emit_phase2()
    warmup(40, wkv[:, 0, 0, 0:TC])           # bridge Q-chain -> attention start
    close(h_kqst, h_hcp, h_xcp, h_wkvp, h_wqp)
    close(h_ps_mm4)
    close(STAT[0][1])
    close(h_persistQ)

    # ====== phase 3: attention (2 heads in flight, DoubleRow everywhere) ======
    ps_sc, h_ps_sc = pool(name="ps_sc", bufs=3, space="PSUM")
    ps_pv, h_ps_pv = pool(name="ps_pv", bufs=1, space="PSUM")
    ptp, h_ptp = pool(name="ptile", bufs=4, side="right")
    stp, h_stp = pool(name="stage", bufs=2, side="right")
    for mp in range(ET):
        heads = [2 * mp, 2 * mp + 1]
        pvs = [ps_pv.tile([HD + 1, TC], F32, tag=f"pv{j}", name="pv")
               for j in range(2)]
        for ktp in range(KT // 2):
            k0 = 2 * ktp
            pts = []
            for j, h in enumerate(heads):
                u_o, a = h // 2, h % 2
                lo = 32 * a
                sc2 = ps_sc.tile([P, 2 * TC], F32, tag="sc2", name="sc2")
                for kk in range(2):
                    nc.tensor.matmul(
                        sc2[:, kk * TC:(kk + 1) * TC],
                        K2[lo:lo + 32, u_o, :, (k0 + kk) * P:(k0 + kk + 1) * P],
                        Q2[lo:lo + 32, u_o, :, :],
                        perf_mode=PM.DoubleRow, skip_group_check=True)
                scv = sc2[:].rearrange("p (kk q) -> p kk q", kk=2)
                ptu = ptp.tile([P, 2, TC], U8, tag="pt2", name="pt2")
                pt8 = ptu[:].bitcast(F8E5)
                # exp split: ACT native Exp on cols [0, QA), DVE bit-trick rest
                nc.scalar.activation(pt8[:, :, 0:QA], scv[:, :, 0:QA],
                                     AF.Exp, scale=0.125)
                nc.vector.tensor_scalar(ptu[:, :, QA:TC], scv[:, :, QA:TC],
                                        scalar1=float(EXP_A), scalar2=float(EXP_B),
                                        op0=OP.mult, op1=OP.add)
                pts.append(pt8)
            for j, h in enumerate(heads):
                nc.tensor.matmul(pvs[j][:], V65[:, k0:k0 + 2, h, 0:HD + 1], pts[j],
                                 start=(ktp == 0), stop=(ktp == KT // 2 - 1),
                                 perf_mode=PM.DoubleRow, skip_group_check=True)
        for j, h in enumerate(heads):
            lo = 64 * (h % 2)
            m = h // 2
            pv = pvs[j]
            den = stp.tile([HD + 1, TC], F32, tag="den", name="den")
            nc.scalar.activation(den[HD:HD + 1, :], pv[HD:HD + 1, :], AF.Copy)
            rb = bcast.tile([P, TC], F32, tag="rb", name="rb")
            recip_bcast(rb[:], den[HD:HD + 1, :], TC)
            stg = stp.tile([HD, TC], F8, tag="stg", name="stg")
            nc.vector.tensor_mul(stg[:], pv[0:HD, :], rb[0:HD, :])
            nc.sync.dma_start(UTs[lo:lo + HD, m, :], stg[:])
    close(h_stp, h_ptp, h_ps_pv, h_ps_sc)
    close(h_big)                     # K/V/Q dead after attention

    # ============ phase 4: output proj + residual + LN2 ============
    STAT[0] = pool(name="ps_stat2", bufs=2, space="PSUM")
    ps_mm, h_ps_mm = pool(name="ps_mm", bufs=4, space="PSUM")
    persistB, h_persistB = pool(name="persistB", bufs=1, side="left")
    x1_sb = persistB.tile([P, ET, TC], F32)
    x1_bf = persistB.tile([P, ET, TC], BF16)
    h2_sb = persistB.tile([P, ET, TC], BF16)
    wpp, h_wpp = pool(name="wproj", bufs=1, side="left")
    wproj = wpp.tile([P, 2, 2, E], F8)
    nc.sync.dma_start(wproj[:], d["d_wproj"].rearrange("u p j o -> p u j o"))

    def warmup2(n, rhs):
        wps = ps_mm.tile([1, rhs.shape[-1]], F32, tag="mm", name="wps")
        for i in range(n):
            nc.tensor.matmul(wps[:], ones_w[:], rhs,
                             start=(i == 0), stop=(i == n - 1),
                             skip_group_check=True)

    warmup2(48, wproj[:, 0, 0, 0:E])         # bridge attention tail -> proj
    mu2_ps = STAT[0][0].tile([1, TC], F32, tag="mu")
    sq2_ps = STAT[0][0].tile([1, TC], F32, tag="sq")
    for m in range(ET):
        pps = ps_mm.tile([P, TC], F32, tag="mm", name="pps")
        for u in range(2):
            nc.tensor.matmul(pps[:], wproj[:, u, :, m * P:(m + 1) * P],
                             UTs[:, 2 * u:2 * u + 2, :],
                             start=(u == 0), stop=(u == 1),
                             perf_mode=PM.DoubleRow)
        # x1 = (proj + bias) + x_slice
        nc.vector.scalar_tensor_tensor(
            x1_sb[:, m, :], pps[:], bproj_sb[:, m:m + 1], xs_sb[:, m, :],
            op0=OP.add, op1=OP.add)
        # LN2 statistics accumulate as each x1 block lands
        nc.scalar.activation(x1_bf[:, m, :], x1_sb[:, m, :], AF.Copy)
        x2 = scratch.tile([P, TC], BF16, tag="ln_x2", name="ln_x2")
        nc.vector.tensor_mul(x2[:], x1_bf[:, m, :], x1_bf[:, m, :])
        nc.tensor.matmul(mu2_ps[:], ones_w[:], x1_bf[:, m, :],
                         start=(m == 0), stop=(m == ET - 1), skip_group_check=True)
        nc.tensor.matmul(sq2_ps[:], ones_w[:], x2[:],
                         start=(m == 0), stop=(m == ET - 1), skip_group_check=True)
    close(h_wpp)
    mu_b2, rs_b2 = ln_chain(mu2_ps, sq2_ps, TC, fast=True)
    ln_apply(h2_sb, x1_bf, mu_b2, rs_b2, TC)

    # ============ phase 5: MLP (bf16 — fp8 is too lossy for the MLP) ============
    mlp, h_mlp = pool(name="mlp", bufs=1, side="left")
    m1_sb = mlp.tile([P, HT, TC], BF16)
    m2_sb = mlp.tile([P, HT, TC], BF16)
    w1p, h_w1p = pool(name="wfc1", bufs=1, side="left")
    wfc1 = w1p.tile([P, ET, H], BF16)
    nc.sync.dma_start(wfc1[:], d["d_wfc1T"].rearrange("(m p) o -> p m o", p=P))
    w3p, h_w3p = pool(name="wfc3", bufs=1, side="left")
    wfc3 = w3p.tile([P, HT, E], BF16)
    nc.sync.dma_start(wfc3[:], d["d_wfc3T"].rearrange("(m p) o -> p m o", p=P))
    w2p, h_w2p = pool(name="wfc2c", bufs=1, side="left")
    wcs = []
    for e in range(HT):
        wc = w2p.tile([P, H], BF16, tag=f"wc{e}", name="wc")
        nc.sync.dma_start(wc[:], d["d_wfc2T"][e * P:(e + 1) * P, :])
        wcs.append(wc)
    warmup2(30, wfc1[:, 0, 0:TC])            # bridge LN2 chain -> fc1
    for m in range(HT):
        ps1 = ps_mm.tile([P, TC], F32, tag="mm", name="ps1")
        for e in range(ET):
            nc.tensor.matmul(ps1[:], wfc1[:, e, m * P:(m + 1) * P],
                             h2_sb[:, e, :], start=(e == 0), stop=(e == ET - 1))
        if m % 2 == 0:
            nc.scalar.activation(m1_sb[:, m, :], ps1[:], AF.Relu,
                                 bias=bfc1_sb[:, m:m + 1])
        else:
            nc.vector.tensor_scalar(m1_sb[:, m, :], ps1[:],
                                    scalar1=bfc1_sb[:, m:m + 1], scalar2=0.0,
                                    op0=OP.add, op1=OP.max)
    close(h_ps_mm, STAT[0][1])

    # fc2: all 16 weight chunks resident -> one dense 256-matmul run
    ps8p, h_ps8p = pool(name="ps8", bufs=6, space="PSUM")
    for m in range(HT):
        psm = ps8p.tile([P, TC], F32, tag="mm8", name="psm")
        for e in range(HT):
            nc.tensor.matmul(psm[:], wcs[e][:, m * P:(m + 1) * P],
                             m1_sb[:, e, :],
                             start=(e == 0), stop=(e == HT - 1),
                             skip_group_check=True)
        if m % 2 == 0:
            nc.scalar.activation(m2_sb[:, m, :], psm[:], AF.Relu,
                                 bias=bfc2_sb[:, m:m + 1])
        else:
            nc.vector.tensor_scalar(m2_sb[:, m, :], psm[:],
                                    scalar1=bfc2_sb[:, m:m + 1], scalar2=0.0,
                                    op0=OP.add, op1=OP.max)
    close(h_ps8p, h_w2p)

    outp, h_outp = pool(name="outp", bufs=2, side="left")
    ps_f3, h_ps_f3 = pool(name="ps_f3", bufs=2, space="PSUM")
    for m in range(ET):
        ps3 = ps_f3.tile([P, TC], F32, tag="f3", name="ps3")
        for e in range(HT):
            nc.tensor.matmul(ps3[:], wfc3[:, e, m * P:(m + 1) * P],
                             m2_sb[:, e, :], start=(e == 0), stop=(e == HT - 1))
        ot = outp.tile([P, TC], F32, tag="ot", name="ot")
        nc.vector.scalar_tensor_tensor(
            ot[:], ps3[:], bfc3_sb[:, m:m + 1], x1_sb[:, m, :],
            op0=OP.add, op1=OP.add)
        nc.sync.dma_start(d["d_outT"][m * P:(m + 1) * P, :], ot[:])
    close(h_ps_f3, h_outp, h_w3p, h_w1p, h_mlp, h_persistB, h_persistA)
    close(_cd)
    close(_c3, _c2, _c1, _c0)


def _host_prep(inputs):
    """Host-side weight permutations / fp8 casts. Returns the common input map."""
    import ml_dtypes
    f8 = ml_dtypes.float8_e4m3
    f32 = np.float32

    qkv_w = np.asarray(inputs["qkv_w"], f32)
    qkv_b = np.asarray(inputs["qkv_b"], f32)
    wq_t = qkv_w[0:E].T.copy()          # [in E, out E]
    wk_t = qkv_w[E:2 * E].T.copy()
    wv_t = qkv_w[2 * E:3 * E].T.copy()
    bq, bk, bv = qkv_b[0:E], qkv_b[E:2 * E], qkv_b[2 * E:3 * E]

    # head-dim pair permutation: m-tile m, col pi = 64a + 32j + p holds
    # feature f = 64*(2m + a) + 2p + j  (head 2m+a, head-dim d = 2p+j)
    pi = np.arange(P)
    colf = np.empty((ET, P), np.int64)
    for m in range(ET):
        colf[m] = (64 * (2 * m + pi // 64) + 2 * (pi % 32)
                   + (pi // 32) % 2)
    colperm = colf.reshape(-1)

    def pair(wt):  # [K, out] -> [K//256, 128, 2, out] DoubleRow stationary
        K = wt.shape[0]
        return np.ascontiguousarray(
            wt.reshape(K // 256, 2, P, wt.shape[1]).transpose(0, 2, 1, 3))

    wk_p = wk_t[:, colperm]
    wq_p = wq_t[:, colperm]
    wkv8 = pair(np.concatenate([wk_p, wv_t], axis=1)).astype(f8)
    wq8 = pair(wq_p).astype(f8)

    proj_w = np.asarray(inputs["proj_w"], f32)
    bproj_eff = np.asarray(inputs["proj_b"], f32) + proj_w @ bv

    x = np.asarray(inputs["x"], f32)[0]          # [T, E]
    ct = lambda a: np.ascontiguousarray(np.asarray(a, f32).T)

    common = {
        "xT": ct(x).astype(ml_dtypes.bfloat16),
        "wkv8": wkv8,
        "wq8": wq8,
        "bqp": np.ascontiguousarray(bq[colperm].reshape(ET, P).T),
        "bkp": np.ascontiguousarray(bk[colperm].reshape(ET, P).T),
        "wproj8": pair(ct(proj_w)).astype(f8),
        "bproj": bproj_eff,
        "wfc1T": ct(inputs["fc1_w"]).astype(ml_dtypes.bfloat16),
        "bfc1": np.asarray(inputs["fc1_b"], f32),
        "wfc2T": ct(inputs["fc2_w"]).astype(ml_dtypes.bfloat16),
        "bfc2": np.asarray(inputs["fc2_b"], f32),
        "wfc3T": ct(inputs["fc3_w"]).astype(ml_dtypes.bfloat16),
        "bfc3": np.asarray(inputs["fc3_b"], f32),
        "lng": np.asarray(inputs["ln_g"], f32),
        "lnb": np.asarray(inputs["ln_b"], f32),
    }
    return common, x, ct


def _get_nc():
    global _BUILT
    if _BUILT is None:
        _BUILT = _build()
    return _BUILT


def run(inputs, trace=False):
    from concourse.bass_utils import run_bass_kernel_spmd

    nc = _get_nc()
    common, x, ct = _host_prep(inputs)
    in_maps = [
        {**common, "xsT": ct(x[c * TC:(c + 1) * TC, :])} for c in range(NCORES)
    ]
    res = run_bass_kernel_spmd(nc, in_maps, core_ids=list(range(NCORES)),
                               trace=trace)
    out = np.empty((1, T, E), np.float32)
    for c in range(NCORES):
        out[0, c * TC:(c + 1) * TC, :] = res.results[c]["outT"].T
    return out, res


def kernel(**inputs) -> np.ndarray:
    out, _ = run(inputs, trace=False)
    return out


# revision 31
# speedup vs baseline: 1.0383x; 1.0383x over previous
"""Trainium2 Bass kernel for a pre-norm transformer block (E=512, H=2048, NH=8, N=4096).

Sharding: sequence-parallel over 8 NeuronCores. Each core computes the full K/V
projection (needs all 4096 tokens) but only its own 512-token slice of queries,
attention output, MLP and residuals. No collectives; host concatenates slices.

v2: every large matmul runs in fp8(e4m3) DoubleRow perf mode (2 contraction
rows per PE cell, 0.5 cycles/row = 4x fewer PE cycles than bf16 at these
shapes). Contraction pairs are realized with host-side weight reshapes
([K, out] -> [K/256, 128, 2, out]); Q/K additionally get a head-dim pair
permutation (d -> (d//2, d%2)) so the 64-wide scores contraction becomes a
[32, 2, *] DoubleRow operand. Softmax exp is split across two engines by
query column: ACT runs native Exp -> fp8e5m2, DVE runs the Schraudolph exp2
bit trick (uint8 = round(0.72135*score + 60) bitcast to e5m2). Each softmax
row (head, q) is served by exactly one engine, so each row's uniform rounding
bias cancels in the softmax normalization (the denominator rides the PV
matmul as a 65th ones-column of V). The V bias is folded into the proj bias
on the host (exact algebra). The residual stream stays fp32.
"""
import sys

sys.path.insert(0, "/opt/trn_rl_repo")
sys.path.insert(0, "/opt/pypackages")

import numpy as np

E, H, NH, HD = 512, 2048, 8, 64
T, NCORES = 4096, 8
TC = T // NCORES          # tokens per core
P = 128
ET = E // P               # 4  feature tiles of E
HT = H // P               # 16 feature tiles of H
KT = T // P               # 32 key-token tiles
EPS = 1e-5
QA = 300                  # q columns per exp tile handled by ACT (rest: DVE)
EXP_A = 0.125 * 4 * np.log2(np.e)   # DVE bit-trick slope (e5m2, scores pre-scaled)
EXP_B = 60.0                        # e5m2 exponent-bias offset (4*15)

_BUILT = None


def _build():
    import concourse.bacc as bacc
    import concourse.mybir as mybir
    import concourse.tile as tile

    dt = mybir.dt
    F32 = dt.float32
    BF16 = dt.bfloat16
    F8 = dt.float8e4

    nc = bacc.Bacc("TRN2", target_bir_lowering=False, debug=False, num_devices=NCORES)

    d = {}
    d["d_xT"] = nc.dram_tensor("xT", [E, T], BF16, kind="ExternalInput").ap()
    d["d_xsT"] = nc.dram_tensor("xsT", [E, TC], F32, kind="ExternalInput").ap()
    # DoubleRow stationary layouts: [n_pair_groups, 128, 2, out_cols]
    d["d_wkv"] = nc.dram_tensor("wkv8", [2, P, 2, 2 * E], F8, kind="ExternalInput").ap()
    d["d_wq"] = nc.dram_tensor("wq8", [2, P, 2, E], F8, kind="ExternalInput").ap()
    d["d_wproj"] = nc.dram_tensor("wproj8", [2, P, 2, E], F8, kind="ExternalInput").ap()
    d["d_wfc1T"] = nc.dram_tensor("wfc1T", [E, H], BF16, kind="ExternalInput").ap()
    d["d_wfc2T"] = nc.dram_tensor("wfc2T", [H, H], BF16, kind="ExternalInput").ap()
    d["d_wfc3T"] = nc.dram_tensor("wfc3T", [H, E], BF16, kind="ExternalInput").ap()
    d["d_bqp"] = nc.dram_tensor("bqp", [P, ET], F32, kind="ExternalInput").ap()
    d["d_bkp"] = nc.dram_tensor("bkp", [P, ET], F32, kind="ExternalInput").ap()
    d["d_bproj"] = nc.dram_tensor("bproj", [E], F32, kind="ExternalInput").ap()
    d["d_bfc1"] = nc.dram_tensor("bfc1", [H], F32, kind="ExternalInput").ap()
    d["d_bfc2"] = nc.dram_tensor("bfc2", [H], F32, kind="ExternalInput").ap()
    d["d_bfc3"] = nc.dram_tensor("bfc3", [E], F32, kind="ExternalInput").ap()
    d["d_lng"] = nc.dram_tensor("lng", [E], F32, kind="ExternalInput").ap()
    d["d_lnb"] = nc.dram_tensor("lnb", [E], F32, kind="ExternalInput").ap()
    d["d_outT"] = nc.dram_tensor("outT", [E, TC], F32, kind="ExternalOutput").ap()

    with tile.TileContext(nc) as tc:
        _emit(nc, tc, tile, mybir, d)

    nc.compile()
    return nc


def _emit(nc, tc, tile, mybir, d):
    dt = mybir.dt
    AF = mybir.ActivationFunctionType
    OP = mybir.AluOpType
    PM = mybir.MatmulPerfMode
    F32 = dt.float32
    BF16 = dt.bfloat16
    F8 = dt.float8e4
    F8E5 = dt.float8e5
    U8 = dt.uint8

    def pool(**kw):
        p = tc.tile_pool(**kw)
        return p.__enter__(), p

    def close(*ps):
        for p in ps:
            p.__exit__(None, None, None)

    # ---- long-lived pools ----
    consts, _c0 = pool(name="consts", bufs=1, side="left")
    stats, _c1 = pool(name="stats", bufs=3, side="left")
    bcast, _c2 = pool(name="bcast", bufs=3, side="left")
    scratch, _c3 = pool(name="scratch", bufs=6, side="left")
    STAT = [pool(name="ps_stat", bufs=2, space="PSUM")]
    drp, _cd = pool(name="drscratch", bufs=4, space="DRAM")

    # ---- constants ----
    onesf = consts.tile([P, 1], F32)
    nc.vector.memset(onesf[:], 1.0)
    ones_w = consts.tile([P, 1], BF16)
    nc.vector.tensor_copy(ones_w[:], onesf[:])
    eps_t = consts.tile([1, 1], F32)
    nc.vector.memset(eps_t[:], EPS)
    eps_p = consts.tile([P, 1], F32)
    nc.vector.memset(eps_p[:], EPS)

    def ld_vec(dram, n, name):  # [n] f32 -> [P, n//P] per-partition layout
        t = consts.tile([P, n // P], F32, name=name)
        nc.sync.dma_start(t[:], dram.rearrange("(m p) -> p m", p=P))
        return t

    g_sb = ld_vec(d["d_lng"], E, "g_sb")
    b_sb = ld_vec(d["d_lnb"], E, "b_sb")
    bq_sb = consts.tile([P, ET], F32, name="bq_sb")
    nc.sync.dma_start(bq_sb[:], d["d_bqp"])
    bk_sb = consts.tile([P, ET], F32, name="bk_sb")
    nc.sync.dma_start(bk_sb[:], d["d_bkp"])
    bproj_sb = ld_vec(d["d_bproj"], E, "bproj_sb")
    bfc1_sb = ld_vec(d["d_bfc1"], H, "bfc1_sb")
    bfc2_sb = ld_vec(d["d_bfc2"], H, "bfc2_sb")
    bfc3_sb = ld_vec(d["d_bfc3"], E, "bfc3_sb")

    def bcast_recip(dst_bb, src_1w, w):
        """dst_bb[P, w] f32 = broadcast(1/src[1, w]) via DRAM stride-0 read +
        post-broadcast reciprocal (all partitions redundantly)."""
        dr = drp.tile([w], F32, tag="drb", name="drb")
        nc.sync.dma_start(dr[None, :], src_1w)
        nc.gpsimd.dma_start(dst_bb, dr[None, :].to_broadcast((P, w)))
        nc.vector.reciprocal(dst_bb, dst_bb)

    def ln_stats(src, w, fast=False):
        """LN statistics from bf16 src tiles [P, ET, w]: (mu_b, rs_b) [P, w] bf16.

        Stats accumulate on the PE; the [1, w] -> [P, w] spread uses gpsimd
        partition_broadcast (no DRAM bounce), and rstd is computed redundantly
        on all partitions after the broadcast.
        """
        x2s = [scratch.tile([P, w], BF16, tag="ln_x2", name="ln_x2")
               for _ in range(ET)]
        for e in range(ET):
            nc.vector.tensor_mul(x2s[e][:], src[:, e, :], src[:, e, :])
        mu_ps = STAT[0][0].tile([1, w], F32, tag="mu")
        sq_ps = STAT[0][0].tile([1, w], F32, tag="sq")
        for e in range(ET):
            nc.tensor.matmul(mu_ps[:], ones_w[:], src[:, e, :],
                             start=(e == 0), stop=(e == ET - 1))
        for e in range(ET):
            nc.tensor.matmul(sq_ps[:], ones_w[:], x2s[e][:],
                             start=(e == 0), stop=(e == ET - 1))
        return ln_chain(mu_ps, sq_ps, w, fast=fast)

    def ln_chain(mu_ps, sq_ps, w, fast=False):
        mu_w = stats.tile([1, w], BF16, tag="mu_w", name="mu_w")
        sq_w = stats.tile([1, w], F32, tag="sq_w", name="sq_w")
        if fast:
            nc.scalar.mul(mu_w[:], mu_ps[:], 1.0 / E)
            nc.scalar.mul(sq_w[:], sq_ps[:], 1.0 / E)
        else:
            nc.vector.tensor_scalar_mul(mu_w[:], mu_ps[:], 1.0 / E)
            nc.vector.tensor_scalar_mul(sq_w[:], sq_ps[:], 1.0 / E)
        mu_b = bcast.tile([P, w], BF16, tag="mu_b", name="mu_b")
        sq_b = bcast.tile([P, w], F32, tag="sq_b", name="sq_b")
        nc.gpsimd.partition_broadcast(mu_b[:], mu_w[:])
        nc.gpsimd.partition_broadcast(sq_b[:], sq_w[:])
        mu2 = scratch.tile([P, w], F32, tag="mu2", name="mu2")
        nc.vector.tensor_mul(mu2[:], mu_b[:], mu_b[:])
        nc.vector.tensor_sub(mu2[:], sq_b[:], mu2[:])
        nc.scalar.activation(mu2[:], mu2[:], AF.Sqrt, bias=eps_p[:])
        rs_b = bcast.tile([P, w], BF16, tag="rs_b", name="rs_b")
        with nc.allow_low_precision(reason="rstd consumed in bf16 either way"):
            nc.vector.reciprocal(rs_b[:], mu2[:])
        return mu_b, rs_b

    def ln_apply(dst8, src, mu_b, rs_b, w, conv_act=False):
        """dst8[P, ET, w] = LN(src bf16); last op converts (one engine/tile)."""
        for e in range(ET):
            t1 = scratch.tile([P, w], BF16, tag="ln_t1", name="ln_t1")
            nc.vector.tensor_sub(t1[:], src[:, e, :], mu_b[:])
            nc.vector.scalar_tensor_tensor(t1[:], t1[:],
                                           g_sb[:, e:e + 1], rs_b[:],
                                           op0=OP.mult, op1=OP.mult)
            if conv_act:
                nc.scalar.activation(dst8[:, e, :], t1[:], AF.Identity,
                                     bias=b_sb[:, e:e + 1])
            else:
                nc.vector.tensor_scalar_add(dst8[:, e, :], t1[:],
                                            scalar1=b_sb[:, e:e + 1])

    # ---- K/V/Q tensors live until end of attention ----
    # K2/Q2 pair layout for DoubleRow scores: head h = 2u+a lives at
    # partitions [32a, 32a+32) of group u; (p, j) -> head dim d = 2p+j.
    big, h_big = pool(name="big", bufs=1, side="right")
    K2 = big.tile([HD, ET, 2, T], F8)           # [part (2 heads), u, j, tok]
    # V + ones column, padded to 80 so dual-fp8 LDWEIGHTS strides are
    # 16B-aligned (per-head 80, per-k-tile 640)
    V65 = big.tile([P, KT, NH, 80], F8)
    Q2 = big.tile([HD, ET, 2, TC], F8)

    # ones column of V65 (denominator rides the PV matmul)
    nc.vector.tensor_copy(
        V65[:, :, :, HD:HD + 1],
        onesf[:, :, None, None].to_broadcast((P, KT, NH, 1)))

    # ====== phase 1: stream x in 512-token chunks, LN, K/V proj ======
    ps_mm4, h_ps_mm4 = pool(name="ps_mm4", bufs=4, space="PSUM")
    CW = 512
    NCH1 = T // CW
    wqp, h_wqp = pool(name="wq", bufs=1, side="right")
    wq = wqp.tile([P, 2, 2, E], F8)             # [part, u_c, j_c, cols]
    wkvp, h_wkvp = pool(name="wkv", bufs=1, side="right")
    wkv = wkvp.tile([P, 2, 2, 2 * E], F8)       # cols: K-perm 512 | V 512
    xcp, h_xcp = pool(name="xc", bufs=3, side="right")
    hcp, h_hcp = pool(name="hc", bufs=3, side="right")

    def warmup(n, rhs, pool_=None):
        wps = (pool_ or ps_mm4).tile([1, rhs.shape[-1]], F32, tag="mm", name="wps")
        for i in range(n):
            nc.tensor.matmul(wps[:], ones_w[:], rhs,
                             start=(i == 0), stop=(i == n - 1),
                             skip_group_check=True)

    kqst, h_kqst = pool(name="kqst", bufs=6, side="right")

    def shuffle_pairs(dst, stg_t, m, cols):
        """Partition-shuffle staging [128, w] fp8 into the pair layout.

        Staging partition pi = 64j + 32a + p holds head (2m+a), d = 2p+j;
        dest is dst[0:64, m, j, cols] (one [64, w] DMA per j, queues split
        between sync and gpsimd)."""
        for j in range(2):
            q = nc.sync if j == 0 else nc.gpsimd
            q.dma_start(dst[0:64, m, j, cols],
                        stg_t[64 * j:64 * j + 64, :])

    def kv_project(hc, ch):
        # K: 4 dense m-tiles; stationary = host-permuted wkv K cols
        for m in range(ET):
            kps = ps_mm4.tile([P, CW], F32, tag="mm", name="kps")
            for u in range(2):
                nc.tensor.matmul(kps[:], wkv[:, u, :, m * P:(m + 1) * P],
                                 hc[:, 2 * u:2 * u + 2, :],
                                 start=(u == 0), stop=(u == 1),
                                 perf_mode=PM.DoubleRow)
            kst = kqst.tile([P, CW], F8, tag="kst", name="kst")
            nc.scalar.activation(kst[:], kps[:], AF.Identity,
                                 bias=bk_sb[:, m:m + 1])
            shuffle_pairs(K2, kst, m, slice(ch * CW, (ch + 1) * CW))
        # V: token-stationary, moving = wkv V cols -> [128 tok, 512 E]
        for t4 in range(CW // P):
            vps = ps_mm4.tile([P, E], F32, tag="mm", name="vps")
            for u in range(2):
                nc.tensor.matmul(vps[:], hc[:, 2 * u:2 * u + 2, t4 * P:(t4 + 1) * P],
                                 wkv[:, u, :, E:2 * E],
                                 start=(u == 0), stop=(u == 1),
                                 perf_mode=PM.DoubleRow)
            kt = ch * (CW // P) + t4
            nc.scalar.activation(
                V65[:, kt, :, 0:HD],
                vps[:].rearrange("p (h d) -> p h d", h=NH), AF.Copy)

    # Q projection resources (emitted mid-pipeline so it overlaps the K/V pass)
    persistA, h_persistA = pool(name="persistA", bufs=1, side="left")
    xs_sb = persistA.tile([P, ET, TC], F32)
    UTs = persistA.tile([P, ET, TC], F8)        # attention out (fp8, feature-major)
    persistQ, h_persistQ = pool(name="persistQ", bufs=1, side="left")
    xs_bf = persistQ.tile([P, ET, TC], BF16)
    hq = persistQ.tile([P, ET, TC], F8)

    def emit_phase2():
        nc.sync.dma_start(xs_sb[:], d["d_xsT"].rearrange("(m p) t -> p m t", p=P))
        for e in range(ET):
            nc.vector.tensor_copy(xs_bf[:, e, :], xs_sb[:, e, :])
        mu_b, rs_b = ln_chain_src(xs_bf, fast=True)
        ln_apply(hq, xs_bf, mu_b, rs_b, TC)
        for m in range(ET):
            qps = ps_mm4.tile([P, TC], F32, tag="mm", name="qps")
            for u in range(2):
                nc.tensor.matmul(qps[:], wq[:, u, :, m * P:(m + 1) * P],
                                 hq[:, 2 * u:2 * u + 2, :],
                                 start=(u == 0), stop=(u == 1),
                                 perf_mode=PM.DoubleRow)
            qst = kqst.tile([P, TC], F8, tag="kst", name="qst")
            nc.scalar.activation(qst[:], qps[:], AF.Identity,
                                 bias=bq_sb[:, m:m + 1])
            shuffle_pairs(Q2, qst, m, slice(0, TC))

    def ln_chain_src(src_bf, fast):
        return ln_stats(src_bf, TC, fast=fast)

    pending = []
    for ch in range(NCH1):
        xc = xcp.tile([P, ET, CW], BF16, tag="xc", name="xc")
        nc.sync.dma_start(
            xc[:],
            d["d_xT"][:, ch * CW:(ch + 1) * CW].rearrange("(m p) t -> p m t", p=P))
        if ch == 0:
            warmup(32, xc[:, 0, 0:256])      # bridge the LN latency at t=0
            nc.sync.dma_start(wkv[:], d["d_wkv"].rearrange("u p j o -> p u j o"))
            nc.sync.dma_start(wq[:], d["d_wq"].rearrange("u p j o -> p u j o"))
        mu_b, rs_b = ln_stats(xc, CW)
        hc = hcp.tile([P, ET, CW], F8, tag="hc", name="hc")
        ln_apply(hc, xc, mu_b, rs_b, CW, conv_act=(ch % 2 == 1))
        pending.append((hc, ch))
        if len(pending) == 2:
            kv_project(*pending.pop(0))
    while pending:
        kv_project(*pending.pop(0))
    emit_phase2()
    warmup(32, wkv[:, 0, 0, 0:256])          # bridge Q-chain -> attention start
    close(h_kqst, h_hcp, h_xcp, h_wkvp, h_wqp)
    close(h_ps_mm4)
    close(STAT[0][1])
    close(h_persistQ)

    # ====== phase 3: attention (2 heads in flight, DoubleRow everywhere) ======
    ps_sc, h_ps_sc = pool(name="ps_sc", bufs=3, space="PSUM")
    ps_pv, h_ps_pv = pool(name="ps_pv", bufs=1, space="PSUM")
    ptp, h_ptp = pool(name="ptile", bufs=4, side="right")
    stp, h_stp = pool(name="stage", bufs=2, side="right")
    for mp in range(ET):
        heads = [2 * mp, 2 * mp + 1]
        pvs = [ps_pv.tile([HD + 1, TC], F32, tag=f"pv{j}", name="pv")
               for j in range(2)]
        for ktp in range(KT // 2):
            k0 = 2 * ktp
            pts = []
            for j, h in enumerate(heads):
                u_o, a = h // 2, h % 2
                lo = 32 * a
                sc2 = ps_sc.tile([P, 2 * TC], F32, tag="sc2", name="sc2")
                for kk in range(2):
                    nc.tensor.matmul(
                        sc2[:, kk * TC:(kk + 1) * TC],
                        K2[lo:lo + 32, u_o, :, (k0 + kk) * P:(k0 + kk + 1) * P],
                        Q2[lo:lo + 32, u_o, :, :],
                        perf_mode=PM.DoubleRow, skip_group_check=True)
                scv = sc2[:].rearrange("p (kk q) -> p kk q", kk=2)
                # exp split into single-producer tiles: ACT native Exp on
                # q cols [0, QA), DVE bit-trick on [QA, TC)
                ptua = ptp.tile([P, 2, QA], U8, tag="pta", name="pta")
                ptub = ptp.tile([P, 2, TC - QA], U8, tag="ptb", name="ptb")
                pt8a = ptua[:].bitcast(F8E5)
                nc.scalar.activation(pt8a, scv[:, :, 0:QA], AF.Exp, scale=0.125)
                nc.vector.tensor_scalar(ptub[:], scv[:, :, QA:TC],
                                        scalar1=float(EXP_A), scalar2=float(EXP_B),
                                        op0=OP.mult, op1=OP.add)
                pts.append((pt8a, ptub[:].bitcast(F8E5)))
            for j, h in enumerate(heads):
                nc.tensor.matmul(pvs[j][:, 0:QA], V65[:, k0:k0 + 2, h, 0:HD + 1],
                                 pts[j][0],
                                 start=(ktp == 0), stop=(ktp == KT // 2 - 1),
                                 perf_mode=PM.DoubleRow, skip_group_check=True)
                nc.tensor.matmul(pvs[j][:, QA:TC], V65[:, k0:k0 + 2, h, 0:HD + 1],
                                 pts[j][1],
                                 start=(ktp == 0), stop=(ktp == KT // 2 - 1),
                                 perf_mode=PM.DoubleRow, skip_group_check=True)
        for j, h in enumerate(heads):
            lo = 64 * (h % 2)
            m = h // 2
            pv = pvs[j]
            den = stp.tile([HD + 1, TC], F32, tag="den", name="den")
            nc.scalar.activation(den[HD:HD + 1, :], pv[HD:HD + 1, :], AF.Copy)
            rb = bcast.tile([P, TC], F32, tag="rb", name="rb")
            bcast_recip(rb[:], den[HD:HD + 1, :], TC)
            stg = stp.tile([HD, TC], F8, tag="stg", name="stg")
            nc.vector.tensor_mul(stg[:], pv[0:HD, :], rb[0:HD, :])
            nc.sync.dma_start(UTs[lo:lo + HD, m, :], stg[:])
    close(h_stp, h_ptp, h_ps_pv, h_ps_sc)
    close(h_big)                     # K/V/Q dead after attention

    # ============ phase 4: output proj + residual + LN2 ============
    STAT[0] = pool(name="ps_stat2", bufs=2, space="PSUM")
    ps_mm, h_ps_mm = pool(name="ps_mm", bufs=4, space="PSUM")
    persistB, h_persistB = pool(name="persistB", bufs=1, side="left")
    x1_sb = persistB.tile([P, ET, TC], F32)
    x1_bf = persistB.tile([P, ET, TC], BF16)
    h2_sb = persistB.tile([P, ET, TC], BF16)
    wpp, h_wpp = pool(name="wproj", bufs=1, side="left")
    wproj = wpp.tile([P, 2, 2, E], F8)
    nc.sync.dma_start(wproj[:], d["d_wproj"].rearrange("u p j o -> p u j o"))

    def warmup2(n, rhs):
        wps = ps_mm.tile([1, rhs.shape[-1]], F32, tag="mm", name="wps")
        for i in range(n):
            nc.tensor.matmul(wps[:], ones_w[:], rhs,
                             start=(i == 0), stop=(i == n - 1),
                             skip_group_check=True)

    warmup2(32, wproj[:, 0, 0, 0:256])       # bridge attention tail -> proj
    mu2_ps = STAT[0][0].tile([1, TC], F32, tag="mu")
    sq2_ps = STAT[0][0].tile([1, TC], F32, tag="sq")
    for m in range(ET):
        pps = ps_mm.tile([P, TC], F32, tag="mm", name="pps")
        for u in range(2):
            nc.tensor.matmul(pps[:], wproj[:, u, :, m * P:(m + 1) * P],
                             UTs[:, 2 * u:2 * u + 2, :],
                             start=(u == 0), stop=(u == 1),
                             perf_mode=PM.DoubleRow)
        # x1 = (proj + bias) + x_slice
        nc.vector.scalar_tensor_tensor(
            x1_sb[:, m, :], pps[:], bproj_sb[:, m:m + 1], xs_sb[:, m, :],
            op0=OP.add, op1=OP.add)
        # LN2 statistics accumulate as each x1 block lands
        nc.scalar.activation(x1_bf[:, m, :], x1_sb[:, m, :], AF.Copy)
        x2 = scratch.tile([P, TC], BF16, tag="ln_x2", name="ln_x2")
        nc.vector.tensor_mul(x2[:], x1_bf[:, m, :], x1_bf[:, m, :])
        nc.tensor.matmul(mu2_ps[:], ones_w[:], x1_bf[:, m, :],
                         start=(m == 0), stop=(m == ET - 1), skip_group_check=True)
        nc.tensor.matmul(sq2_ps[:], ones_w[:], x2[:],
                         start=(m == 0), stop=(m == ET - 1), skip_group_check=True)
    close(h_wpp)
    mu_b2, rs_b2 = ln_chain(mu2_ps, sq2_ps, TC, fast=True)
    ln_apply(h2_sb, x1_bf, mu_b2, rs_b2, TC)

    # ============ phase 5: MLP (bf16 — fp8 is too lossy for the MLP) ============
    mlp, h_mlp = pool(name="mlp", bufs=1, side="left")
    m1_sb = mlp.tile([P, HT, TC], BF16)
    m2_sb = mlp.tile([P, HT, TC], BF16)
    w1p, h_w1p = pool(name="wfc1", bufs=1, side="left")
    wfc1 = w1p.tile([P, ET, H], BF16)
    nc.sync.dma_start(wfc1[:], d["d_wfc1T"].rearrange("(m p) o -> p m o", p=P))
    w3p, h_w3p = pool(name="wfc3", bufs=1, side="left")
    wfc3 = w3p.tile([P, HT, E], BF16)
    nc.sync.dma_start(wfc3[:], d["d_wfc3T"].rearrange("(m p) o -> p m o", p=P))
    w2p, h_w2p = pool(name="wfc2c", bufs=1, side="left")
    wcs = []
    for e in range(HT):
        wc = w2p.tile([P, H], BF16, tag=f"wc{e}", name="wc")
        nc.sync.dma_start(wc[:], d["d_wfc2T"][e * P:(e + 1) * P, :])
        wcs.append(wc)
    warmup2(24, wfc1[:, 0, 0:256])           # bridge LN2 chain -> fc1
    for m in range(HT):
        ps1 = ps_mm.tile([P, TC], F32, tag="mm", name="ps1")
        for e in range(ET):
            nc.tensor.matmul(ps1[:], wfc1[:, e, m * P:(m + 1) * P],
                             h2_sb[:, e, :], start=(e == 0), stop=(e == ET - 1))
        nc.scalar.activation(m1_sb[:, m, :], ps1[:], AF.Relu,
                             bias=bfc1_sb[:, m:m + 1])
    close(h_ps_mm, STAT[0][1])

    # fc2: all 16 weight chunks resident -> one dense 256-matmul run
    ps8p, h_ps8p = pool(name="ps8", bufs=6, space="PSUM")
    for m in range(HT):
        psm = ps8p.tile([P, TC], F32, tag="mm8", name="psm")
        for e in range(HT):
            nc.tensor.matmul(psm[:], wcs[e][:, m * P:(m + 1) * P],
                             m1_sb[:, e, :],
                             start=(e == 0), stop=(e == HT - 1),
                             skip_group_check=True)
        nc.vector.tensor_scalar(m2_sb[:, m, :], psm[:],
                                scalar1=bfc2_sb[:, m:m + 1], scalar2=0.0,
                                op0=OP.add, op1=OP.max)
    close(h_ps8p, h_w2p)

    outp, h_outp = pool(name="outp", bufs=2, side="left")
    ps_f3, h_ps_f3 = pool(name="ps_f3", bufs=2, space="PSUM")
    for m in range(ET):
        ps3 = ps_f3.tile([P, TC], F32, tag="f3", name="ps3")
        for e in range(HT):
            nc.tensor.matmul(ps3[:], wfc3[:, e, m * P:(m + 1) * P],
                             m2_sb[:, e, :], start=(e == 0), stop=(e == HT - 1))
        ot = outp.tile([P, TC], F32, tag="ot", name="ot")
        nc.vector.scalar_tensor_tensor(
            ot[:], ps3[:], bfc3_sb[:, m:m + 1], x1_sb[:, m, :],
            op0=OP.add, op1=OP.add)
        nc.sync.dma_start(d["d_outT"][m * P:(m + 1) * P, :], ot[:])
    close(h_ps_f3, h_outp, h_w3p, h_w1p, h_mlp, h_persistB, h_persistA)
    close(_cd)
    close(_c3, _c2, _c1, _c0)


def _host_prep(inputs):
    """Host-side weight permutations / fp8 casts. Returns the common input map."""
    import ml_dtypes
    f8 = ml_dtypes.float8_e4m3
    f32 = np.float32

    qkv_w = np.asarray(inputs["qkv_w"], f32)
    qkv_b = np.asarray(inputs["qkv_b"], f32)
    wq_t = qkv_w[0:E].T.copy()          # [in E, out E]
    wk_t = qkv_w[E:2 * E].T.copy()
    wv_t = qkv_w[2 * E:3 * E].T.copy()
    bq, bk, bv = qkv_b[0:E], qkv_b[E:2 * E], qkv_b[2 * E:3 * E]

    # head-dim pair permutation: m-tile m, col pi = 64j + 32a + p holds
    # feature f = 64*(2m + a) + 2p + j  (head 2m+a, head-dim d = 2p+j)
    pi = np.arange(P)
    colf = np.empty((ET, P), np.int64)
    for m in range(ET):
        colf[m] = (64 * (2 * m + (pi // 32) % 2) + 2 * (pi % 32)
                   + pi // 64)
    colperm = colf.reshape(-1)

    def pair(wt):  # [K, out] -> [K//256, 128, 2, out] DoubleRow stationary
        K = wt.shape[0]
        return np.ascontiguousarray(
            wt.reshape(K // 256, 2, P, wt.shape[1]).transpose(0, 2, 1, 3))

    wk_p = wk_t[:, colperm]
    wq_p = wq_t[:, colperm]
    wkv8 = pair(np.concatenate([wk_p, wv_t], axis=1)).astype(f8)
    wq8 = pair(wq_p).astype(f8)

    proj_w = np.asarray(inputs["proj_w"], f32)
    bproj_eff = np.asarray(inputs["proj_b"], f32) + proj_w @ bv

    x = np.asarray(inputs["x"], f32)[0]          # [T, E]
    ct = lambda a: np.ascontiguousarray(np.asarray(a, f32).T)

    common = {
        "xT": ct(x).astype(ml_dtypes.bfloat16),
        "wkv8": wkv8,
        "wq8": wq8,
        "bqp": np.ascontiguousarray(bq[colperm].reshape(ET, P).T),
        "bkp": np.ascontiguousarray(bk[colperm].reshape(ET, P).T),
        "wproj8": pair(ct(proj_w)).astype(f8),
        "bproj": bproj_eff,
        "wfc1T": ct(inputs["fc1_w"]).astype(ml_dtypes.bfloat16),
        "bfc1": np.asarray(inputs["fc1_b"], f32),
        "wfc2T": ct(inputs["fc2_w"]).astype(ml_dtypes.bfloat16),
        "bfc2": np.asarray(inputs["fc2_b"], f32),
        "wfc3T": ct(inputs["fc3_w"]).astype(ml_dtypes.bfloat16),
        "bfc3": np.asarray(inputs["fc3_b"], f32),
        "lng": np.asarray(inputs["ln_g"], f32),
        "lnb": np.asarray(inputs["ln_b"], f32),
    }
    return common, x, ct


def _get_nc():
    global _BUILT
    if _BUILT is None:
        _BUILT = _build()
    return _BUILT


def run(inputs, trace=False):
    from concourse.bass_utils import run_bass_kernel_spmd

    nc = _get_nc()
    common, x, ct = _host_prep(inputs)
    in_maps = [
        {**common, "xsT": ct(x[c * TC:(c + 1) * TC, :])} for c in range(NCORES)
    ]
    res = run_bass_kernel_spmd(nc, in_maps, core_ids=list(range(NCORES)),
                               trace=trace)
    out = np.empty((1, T, E), np.float32)
    for c in range(NCORES):
        out[0, c * TC:(c + 1) * TC, :] = res.results[c]["outT"].T
    return out, res


def kernel(**inputs) -> np.ndarray:
    out, _ = run(inputs, trace=False)
    return out


# revision 37
# speedup vs baseline: 1.1449x; 1.1027x over previous
"""Trainium2 Bass kernel for a pre-norm transformer block (E=512, H=2048, NH=8, N=4096).

Sharding: sequence-parallel over 8 NeuronCores. Each core computes the full K/V
projection (needs all 4096 tokens) but only its own 512-token slice of queries,
attention output, MLP and residuals. No collectives; host concatenates slices.

v2: every large matmul runs in fp8(e4m3) DoubleRow perf mode (2 contraction
rows per PE cell, 0.5 cycles/row = 4x fewer PE cycles than bf16 at these
shapes). Contraction pairs are realized with host-side weight reshapes
([K, out] -> [K/256, 128, 2, out]); Q/K additionally get a head-dim pair
permutation (d -> (d//2, d%2)) so the 64-wide scores contraction becomes a
[32, 2, *] DoubleRow operand. Softmax exp is split across two engines by
query column: ACT runs native Exp -> fp8e5m2, DVE runs the Schraudolph exp2
bit trick (uint8 = round(0.72135*score + 60) bitcast to e5m2). Each softmax
row (head, q) is served by exactly one engine, so each row's uniform rounding
bias cancels in the softmax normalization (the denominator rides the PV
matmul as a 65th ones-column of V). The V bias is folded into the proj bias
on the host (exact algebra). The residual stream stays fp32.
"""
import sys

sys.path.insert(0, "/opt/trn_rl_repo")
sys.path.insert(0, "/opt/pypackages")

import numpy as np

E, H, NH, HD = 512, 2048, 8, 64
T, NCORES = 4096, 8
TC = T // NCORES          # tokens per core
P = 128
ET = E // P               # 4  feature tiles of E
HT = H // P               # 16 feature tiles of H
KT = T // P               # 32 key-token tiles
EPS = 1e-5
QA = 264                  # q columns per exp tile handled by ACT (rest: DVE)
EXP_A = 0.125 * 4 * np.log2(np.e)   # DVE bit-trick slope (e5m2, scores pre-scaled)
EXP_B = 60.0                        # e5m2 exponent-bias offset (4*15)

_BUILT = None


def _build():
    import concourse.bacc as bacc
    import concourse.mybir as mybir
    import concourse.tile as tile

    dt = mybir.dt
    F32 = dt.float32
    BF16 = dt.bfloat16
    F8 = dt.float8e4

    nc = bacc.Bacc("TRN2", target_bir_lowering=False, debug=False, num_devices=NCORES)

    d = {}
    d["d_xT"] = nc.dram_tensor("xT", [E, T], BF16, kind="ExternalInput").ap()
    d["d_xsT"] = nc.dram_tensor("xsT", [E, TC], F32, kind="ExternalInput").ap()
    # DoubleRow stationary layouts: [n_pair_groups, 128, 2, out_cols]
    d["d_wkv"] = nc.dram_tensor("wkv8", [2, P, 2, 2 * E], F8, kind="ExternalInput").ap()
    d["d_wq"] = nc.dram_tensor("wq8", [2, P, 2, E], F8, kind="ExternalInput").ap()
    d["d_wproj"] = nc.dram_tensor("wproj8", [2, P, 2, E], F8, kind="ExternalInput").ap()
    d["d_wfc1T"] = nc.dram_tensor("wfc1T", [E, H], BF16, kind="ExternalInput").ap()
    d["d_wfc2T"] = nc.dram_tensor("wfc2T", [H, H], BF16, kind="ExternalInput").ap()
    d["d_wfc3T"] = nc.dram_tensor("wfc3T", [H, E], BF16, kind="ExternalInput").ap()
    d["d_bqp"] = nc.dram_tensor("bqp", [P, ET], F32, kind="ExternalInput").ap()
    d["d_bkp"] = nc.dram_tensor("bkp", [P, ET], F32, kind="ExternalInput").ap()
    d["d_bproj"] = nc.dram_tensor("bproj", [E], F32, kind="ExternalInput").ap()
    d["d_bfc1"] = nc.dram_tensor("bfc1", [H], F32, kind="ExternalInput").ap()
    d["d_bfc2"] = nc.dram_tensor("bfc2", [H], F32, kind="ExternalInput").ap()
    d["d_bfc3"] = nc.dram_tensor("bfc3", [E], F32, kind="ExternalInput").ap()
    d["d_lng"] = nc.dram_tensor("lng", [E], F32, kind="ExternalInput").ap()
    d["d_lnb"] = nc.dram_tensor("lnb", [E], F32, kind="ExternalInput").ap()
    d["d_outT"] = nc.dram_tensor("outT", [E, TC], F32, kind="ExternalOutput").ap()

    with tile.TileContext(nc) as tc:
        _emit(nc, tc, tile, mybir, d)

    nc.compile()
    return nc


def _emit(nc, tc, tile, mybir, d):
    dt = mybir.dt
    AF = mybir.ActivationFunctionType
    OP = mybir.AluOpType
    PM = mybir.MatmulPerfMode
    F32 = dt.float32
    BF16 = dt.bfloat16
    F8 = dt.float8e4
    F8E5 = dt.float8e5
    U8 = dt.uint8

    def pool(**kw):
        p = tc.tile_pool(**kw)
        return p.__enter__(), p

    def close(*ps):
        for p in ps:
            p.__exit__(None, None, None)

    # ---- long-lived pools ----
    consts, _c0 = pool(name="consts", bufs=1, side="left")
    stats, _c1 = pool(name="stats", bufs=3, side="left")
    bcast, _c2 = pool(name="bcast", bufs=2, side="left")
    scratch, _c3 = pool(name="scratch", bufs=4, side="left")
    STAT = [pool(name="ps_stat", bufs=2, space="PSUM")]
    drp, _cd = pool(name="drscratch", bufs=4, space="DRAM")

    # ---- constants ----
    onesf = consts.tile([P, 1], F32)
    nc.vector.memset(onesf[:], 1.0)
    ones_w = consts.tile([P, 1], BF16)
    nc.vector.tensor_copy(ones_w[:], onesf[:])
    eps_t = consts.tile([1, 1], F32)
    nc.vector.memset(eps_t[:], EPS)
    eps_p = consts.tile([P, 1], F32)
    nc.vector.memset(eps_p[:], EPS)

    def ld_vec(dram, n, name):  # [n] f32 -> [P, n//P] per-partition layout
        t = consts.tile([P, n // P], F32, name=name)
        nc.sync.dma_start(t[:], dram.rearrange("(m p) -> p m", p=P))
        return t

    g_sb = ld_vec(d["d_lng"], E, "g_sb")
    b_sb = ld_vec(d["d_lnb"], E, "b_sb")
    bq_sb = consts.tile([P, ET], F32, name="bq_sb")
    nc.sync.dma_start(bq_sb[:], d["d_bqp"])
    bk_sb = consts.tile([P, ET], F32, name="bk_sb")
    nc.sync.dma_start(bk_sb[:], d["d_bkp"])
    bproj_sb = ld_vec(d["d_bproj"], E, "bproj_sb")
    bfc1_sb = ld_vec(d["d_bfc1"], H, "bfc1_sb")
    bfc2_sb = ld_vec(d["d_bfc2"], H, "bfc2_sb")
    bfc3_sb = ld_vec(d["d_bfc3"], E, "bfc3_sb")

    def bcast_recip(dst_bb, src_1w, w):
        """dst_bb[P, w] f32 = broadcast(1/src[1, w]) via DRAM stride-0 read +
        post-broadcast approx reciprocal (all partitions redundantly)."""
        dr = drp.tile([w], F32, tag="drb", name="drb")
        nc.sync.dma_start(dr[None, :], src_1w)
        nc.gpsimd.dma_start(dst_bb, dr[None, :].to_broadcast((P, w)))
        nc.vector.reciprocal_approx_fast(out=dst_bb, in_=dst_bb)

    def ln_stats(src, w, fast=False):
        """LN statistics from bf16 src tiles [P, ET, w]: (mu_b, rs_b) [P, w] bf16.

        Stats accumulate on the PE; the [1, w] -> [P, w] spread uses gpsimd
        partition_broadcast (no DRAM bounce), and rstd is computed redundantly
        on all partitions after the broadcast.
        """
        x2s = [scratch.tile([P, w], BF16, tag="ln_x2", name="ln_x2")
               for _ in range(ET)]
        for e in range(ET):
            nc.vector.tensor_mul(x2s[e][:], src[:, e, :], src[:, e, :])
        mu_ps = STAT[0][0].tile([1, w], F32, tag="mu")
        sq_ps = STAT[0][0].tile([1, w], F32, tag="sq")
        for e in range(ET):
            nc.tensor.matmul(mu_ps[:], ones_w[:], src[:, e, :],
                             start=(e == 0), stop=(e == ET - 1))
        for e in range(ET):
            nc.tensor.matmul(sq_ps[:], ones_w[:], x2s[e][:],
                             start=(e == 0), stop=(e == ET - 1))
        return ln_chain(mu_ps, sq_ps, w, fast=fast)

    def ln_chain(mu_ps, sq_ps, w, fast=False):
        # mean kept f32 on the fast path (x1 means are larger; bf16 there
        # costs visible accuracy); eps folds into the E[x^2] scale op
        mdt = F32 if fast else BF16
        mu_w = stats.tile([1, w], mdt, tag="mu_w", name="mu_w")
        sq_w = stats.tile([1, w], F32, tag="sq_w", name="sq_w")
        nc.vector.tensor_scalar_mul(mu_w[:], mu_ps[:], 1.0 / E)
        nc.scalar.activation(sq_w[:], sq_ps[:], AF.Identity, bias=eps_t[:],
                             scale=1.0 / E)
        mu_b = bcast.tile([P, w], mdt, tag="mu_b", name="mu_b")
        sq_b = bcast.tile([P, w], F32, tag="sq_b", name="sq_b")
        nc.gpsimd.partition_broadcast(mu_b[:], mu_w[:])
        nc.gpsimd.partition_broadcast(sq_b[:], sq_w[:])
        mu2 = scratch.tile([P, w], F32, tag="mu2", name="mu2")
        nc.vector.tensor_mul(mu2[:], mu_b[:], mu_b[:])
        nc.vector.tensor_sub(mu2[:], sq_b[:], mu2[:])
        nc.vector.reciprocal_approx_fast(out=mu2[:], in_=mu2[:])
        rs_b = bcast.tile([P, w], BF16, tag="rs_b", name="rs_b")
        nc.scalar.activation(rs_b[:], mu2[:], AF.Sqrt)
        return mu_b, rs_b

    def ln_apply(dst8, src, mu_b, rs_b, w, conv_act=False):
        """dst8[P, ET, w] = LN(src bf16); last op converts (one engine/tile)."""
        for e in range(ET):
            t1 = scratch.tile([P, w], BF16, tag="ln_t1", name="ln_t1")
            nc.vector.tensor_sub(t1[:], src[:, e, :], mu_b[:])
            nc.vector.scalar_tensor_tensor(t1[:], t1[:],
                                           g_sb[:, e:e + 1], rs_b[:],
                                           op0=OP.mult, op1=OP.mult)
            if conv_act:
                nc.scalar.activation(dst8[:, e, :], t1[:], AF.Identity,
                                     bias=b_sb[:, e:e + 1])
            else:
                nc.vector.tensor_scalar_add(dst8[:, e, :], t1[:],
                                            scalar1=b_sb[:, e:e + 1])

    # ---- K/V/Q tensors live until end of attention ----
    # K2/Q2 pair layout for DoubleRow scores: head h = 2u+a lives at
    # partitions [32a, 32a+32) of group u; (p, j) -> head dim d = 2p+j.
    big, h_big = pool(name="big", bufs=1, side="right")
    K2 = big.tile([HD, ET, 2, T], F8)           # [part (2 heads), u, j, tok]
    # V + ones column, padded to 80 so dual-fp8 LDWEIGHTS strides are
    # 16B-aligned (per-head 80, per-k-tile 640)
    V65 = big.tile([P, KT, NH, 80], F8)
    Q2 = big.tile([HD, ET, 2, TC], F8)

    # ones column of V65 (denominator rides the PV matmul)
    nc.vector.tensor_copy(
        V65[:, :, :, HD:HD + 1],
        onesf[:, :, None, None].to_broadcast((P, KT, NH, 1)))

    # ====== phase 1: stream x in 512-token chunks, LN, K/V proj ======
    ps_mm4, h_ps_mm4 = pool(name="ps_mm4", bufs=4, space="PSUM")
    CW = 512
    NCH1 = T // CW
    wqp, h_wqp = pool(name="wq", bufs=1, side="right")
    wq = wqp.tile([P, 2, 2, E], F8)             # [part, u_c, j_c, cols]
    wkvp, h_wkvp = pool(name="wkv", bufs=1, side="right")
    wkv = wkvp.tile([P, 2, 2, 2 * E], F8)       # cols: K-perm 512 | V 512
    xcp, h_xcp = pool(name="xc", bufs=3, side="right")
    hcp, h_hcp = pool(name="hc", bufs=3, side="right")

    def warmup(n, rhs, pool_=None):
        wps = (pool_ or ps_mm4).tile([1, rhs.shape[-1]], F32, tag="mm", name="wps")
        for i in range(n):
            nc.tensor.matmul(wps[:], ones_w[:], rhs,
                             start=(i == 0), stop=(i == n - 1),
                             skip_group_check=True)

    kqst, h_kqst = pool(name="kqst", bufs=6, side="right")

    def shuffle_pairs(dst, stg_t, m, cols):
        """Partition-shuffle staging [128, w] fp8 into the pair layout.

        Staging partition pi = 64j + 32a + p holds head (2m+a), d = 2p+j;
        dest is dst[0:64, m, j, cols] (one [64, w] DMA per j, queues split
        between sync and gpsimd)."""
        for j in range(2):
            q = nc.sync if j == 0 else nc.gpsimd
            q.dma_start(dst[0:64, m, j, cols],
                        stg_t[64 * j:64 * j + 64, :])

    def kv_project(hc, ch):
        # K: 4 dense m-tiles; stationary = host-permuted wkv K cols
        for m in range(ET):
            kps = ps_mm4.tile([P, CW], F32, tag="mm", name="kps")
            for u in range(2):
                nc.tensor.matmul(kps[:], wkv[:, u, :, m * P:(m + 1) * P],
                                 hc[:, 2 * u:2 * u + 2, :],
                                 start=(u == 0), stop=(u == 1),
                                 perf_mode=PM.DoubleRow)
            kst = kqst.tile([P, CW], F8, tag="kst", name="kst")
            nc.scalar.activation(kst[:], kps[:], AF.Identity,
                                 bias=bk_sb[:, m:m + 1])
            shuffle_pairs(K2, kst, m, slice(ch * CW, (ch + 1) * CW))
        # V: token-stationary, moving = wkv V cols -> [128 tok, 512 E]
        for t4 in range(CW // P):
            vps = ps_mm4.tile([P, E], F32, tag="mm", name="vps")
            for u in range(2):
                nc.tensor.matmul(vps[:], hc[:, 2 * u:2 * u + 2, t4 * P:(t4 + 1) * P],
                                 wkv[:, u, :, E:2 * E],
                                 start=(u == 0), stop=(u == 1),
                                 perf_mode=PM.DoubleRow)
            kt = ch * (CW // P) + t4
            if ch % 2 == 0:
                nc.scalar.activation(
                    V65[:, kt, :, 0:HD],
                    vps[:].rearrange("p (h d) -> p h d", h=NH), AF.Copy)
            else:
                nc.vector.tensor_copy(
                    V65[:, kt, :, 0:HD],
                    vps[:].rearrange("p (h d) -> p h d", h=NH))

    # Q projection resources (emitted mid-pipeline so it overlaps the K/V pass)
    persistA, h_persistA = pool(name="persistA", bufs=1, side="left")
    xs_sb = persistA.tile([P, ET, TC], F32)
    UTs = persistA.tile([P, ET, TC], F8)        # attention out (fp8, feature-major)
    persistQ, h_persistQ = pool(name="persistQ", bufs=1, side="left")
    xs_bf = persistQ.tile([P, ET, TC], BF16)
    hq = persistQ.tile([P, ET, TC], F8)

    def emit_phase2():
        nc.sync.dma_start(xs_sb[:], d["d_xsT"].rearrange("(m p) t -> p m t", p=P))
        for e in range(ET):
            nc.vector.tensor_copy(xs_bf[:, e, :], xs_sb[:, e, :])
        mu_b, rs_b = ln_chain_src(xs_bf, fast=True)
        ln_apply(hq, xs_bf, mu_b, rs_b, TC)
        for m in range(ET):
            qps = ps_mm4.tile([P, TC], F32, tag="mm", name="qps")
            for u in range(2):
                nc.tensor.matmul(qps[:], wq[:, u, :, m * P:(m + 1) * P],
                                 hq[:, 2 * u:2 * u + 2, :],
                                 start=(u == 0), stop=(u == 1),
                                 perf_mode=PM.DoubleRow)
            qst = kqst.tile([P, TC], F8, tag="kst", name="qst")
            nc.scalar.activation(qst[:], qps[:], AF.Identity,
                                 bias=bq_sb[:, m:m + 1])
            shuffle_pairs(Q2, qst, m, slice(0, TC))

    def ln_chain_src(src_bf, fast):
        return ln_stats(src_bf, TC, fast=fast)

    pending = []
    for ch in range(NCH1):
        xc = xcp.tile([P, ET, CW], BF16, tag="xc", name="xc")
        nc.sync.dma_start(
            xc[:],
            d["d_xT"][:, ch * CW:(ch + 1) * CW].rearrange("(m p) t -> p m t", p=P))
        if ch == 0:
            warmup(32, xc[:, 0, 0:256])      # bridge the LN latency at t=0
            nc.sync.dma_start(wkv[:], d["d_wkv"].rearrange("u p j o -> p u j o"))
            nc.sync.dma_start(wq[:], d["d_wq"].rearrange("u p j o -> p u j o"))
        mu_b, rs_b = ln_stats(xc, CW)
        hc = hcp.tile([P, ET, CW], F8, tag="hc", name="hc")
        ln_apply(hc, xc, mu_b, rs_b, CW, conv_act=(ch % 2 == 1))
        pending.append((hc, ch))
        if len(pending) == 2:
            kv_project(*pending.pop(0))
    while pending:
        kv_project(*pending.pop(0))
    emit_phase2()
    warmup(32, wkv[:, 0, 0, 0:256])          # bridge Q-chain -> attention start
    close(h_kqst, h_hcp, h_xcp, h_wkvp, h_wqp)
    close(h_ps_mm4)
    close(STAT[0][1])
    close(h_persistQ)

    # ====== phase 3: attention (2 heads in flight, DoubleRow everywhere) ======
    # Software-pipelined: PV(ktp) is emitted AFTER scores(ktp+1) so the
    # in-order PE queue never head-of-line blocks on the exp chain.
    ps_sc, h_ps_sc = pool(name="ps_sc", bufs=3, space="PSUM")
    ps_pv, h_ps_pv = pool(name="ps_pv", bufs=1, space="PSUM")
    ptp, h_ptp = pool(name="ptile", bufs=6, side="right")
    stp, h_stp = pool(name="stage", bufs=2, side="right")
    for mp in range(ET):
        heads = [2 * mp, 2 * mp + 1]
        pvs = [ps_pv.tile([HD + 1, TC], F32, tag=f"pv{j}", name="pv")
               for j in range(2)]

        def emit_scores(ktp):
            k0 = 2 * ktp
            pts = []
            for j, h in enumerate(heads):
                u_o, a = h // 2, h % 2
                lo = 32 * a
                sc2 = ps_sc.tile([P, 2 * TC], F32, tag="sc2", name="sc2")
                for kk in range(2):
                    nc.tensor.matmul(
                        sc2[:, kk * TC:(kk + 1) * TC],
                        K2[lo:lo + 32, u_o, :, (k0 + kk) * P:(k0 + kk + 1) * P],
                        Q2[lo:lo + 32, u_o, :, :],
                        perf_mode=PM.DoubleRow, skip_group_check=True)
                scv = sc2[:].rearrange("p (kk q) -> p kk q", kk=2)
                # exp split into single-producer tiles: ACT native Exp on
                # q cols [0, QA), DVE bit-trick on [QA, TC)
                ptua = ptp.tile([P, 2, QA], U8, tag="pta", name="pta")
                ptub = ptp.tile([P, 2, TC - QA], U8, tag="ptb", name="ptb")
                pt8a = ptua[:].bitcast(F8E5)
                nc.scalar.activation(pt8a, scv[:, :, 0:QA], AF.Exp, scale=0.125)
                nc.vector.tensor_scalar(ptub[:], scv[:, :, QA:TC],
                                        scalar1=float(EXP_A), scalar2=float(EXP_B),
                                        op0=OP.mult, op1=OP.add)
                pts.append((pt8a, ptub[:].bitcast(F8E5)))
            return pts

        def emit_pv(ktp, pts):
            k0 = 2 * ktp
            for j, h in enumerate(heads):
                nc.tensor.matmul(pvs[j][:, 0:QA], V65[:, k0:k0 + 2, h, 0:HD + 1],
                                 pts[j][0],
                                 start=(ktp == 0), stop=(ktp == KT // 2 - 1),
                                 perf_mode=PM.DoubleRow, skip_group_check=True)
                nc.tensor.matmul(pvs[j][:, QA:TC], V65[:, k0:k0 + 2, h, 0:HD + 1],
                                 pts[j][1],
                                 start=(ktp == 0), stop=(ktp == KT // 2 - 1),
                                 perf_mode=PM.DoubleRow, skip_group_check=True)

        prev = None
        for ktp in range(KT // 2):
            pts = emit_scores(ktp)
            if prev is not None:
                emit_pv(ktp - 1, prev)
            prev = pts
        emit_pv(KT // 2 - 1, prev)
        for j, h in enumerate(heads):
            lo = 64 * (h % 2)
            m = h // 2
            pv = pvs[j]
            den = stp.tile([HD + 1, TC], F32, tag="den", name="den")
            nc.scalar.activation(den[HD:HD + 1, :], pv[HD:HD + 1, :], AF.Copy)
            rb = bcast.tile([P, TC], F32, tag="rb", name="rb")
            bcast_recip(rb[:], den[HD:HD + 1, :], TC)
            stg = stp.tile([HD, TC], F8, tag="stg", name="stg")
            nc.vector.tensor_mul(stg[:], pv[0:HD, :], rb[0:HD, :])
            nc.sync.dma_start(UTs[lo:lo + HD, m, :], stg[:])
    close(h_stp, h_ptp, h_ps_pv, h_ps_sc)
    close(h_big)                     # K/V/Q dead after attention

    # ============ phase 4: output proj + residual + LN2 ============
    STAT[0] = pool(name="ps_stat2", bufs=2, space="PSUM")
    ps_mm, h_ps_mm = pool(name="ps_mm", bufs=4, space="PSUM")
    persistB, h_persistB = pool(name="persistB", bufs=1, side="left")
    x1_sb = persistB.tile([P, ET, TC], F32)
    x1_bf = persistB.tile([P, ET, TC], BF16)
    h2_sb = persistB.tile([P, ET, TC], BF16)
    wpp, h_wpp = pool(name="wproj", bufs=1, side="left")
    wproj = wpp.tile([P, 2, 2, E], F8)
    nc.sync.dma_start(wproj[:], d["d_wproj"].rearrange("u p j o -> p u j o"))

    def warmup2(n, rhs):
        wps = ps_mm.tile([1, rhs.shape[-1]], F32, tag="mm", name="wps")
        for i in range(n):
            nc.tensor.matmul(wps[:], ones_w[:], rhs,
                             start=(i == 0), stop=(i == n - 1),
                             skip_group_check=True)

    warmup2(32, wproj[:, 0, 0, 0:256])       # bridge attention tail -> proj
    mu2_ps = STAT[0][0].tile([1, TC], F32, tag="mu")
    sq2_ps = STAT[0][0].tile([1, TC], F32, tag="sq")
    for m in range(ET):
        pps = ps_mm.tile([P, TC], F32, tag="mm", name="pps")
        for u in range(2):
            nc.tensor.matmul(pps[:], wproj[:, u, :, m * P:(m + 1) * P],
                             UTs[:, 2 * u:2 * u + 2, :],
                             start=(u == 0), stop=(u == 1),
                             perf_mode=PM.DoubleRow)
        # x1 = (proj + bias) + x_slice
        nc.vector.scalar_tensor_tensor(
            x1_sb[:, m, :], pps[:], bproj_sb[:, m:m + 1], xs_sb[:, m, :],
            op0=OP.add, op1=OP.add)
        # LN2 statistics accumulate as each x1 block lands
        nc.scalar.activation(x1_bf[:, m, :], x1_sb[:, m, :], AF.Copy)
        x2 = scratch.tile([P, TC], BF16, tag="ln_x2", name="ln_x2")
        nc.vector.tensor_mul(x2[:], x1_bf[:, m, :], x1_bf[:, m, :])
        nc.tensor.matmul(mu2_ps[:], ones_w[:], x1_bf[:, m, :],
                         start=(m == 0), stop=(m == ET - 1), skip_group_check=True)
        nc.tensor.matmul(sq2_ps[:], ones_w[:], x2[:],
                         start=(m == 0), stop=(m == ET - 1), skip_group_check=True)
    close(h_wpp)
    mu_b2, rs_b2 = ln_chain(mu2_ps, sq2_ps, TC, fast=True)
    ln_apply(h2_sb, x1_bf, mu_b2, rs_b2, TC)

    # ============ phase 5: MLP (bf16 — fp8 is too lossy for the MLP) ============
    mlp, h_mlp = pool(name="mlp", bufs=1, side="left")
    m1_sb = mlp.tile([P, HT, TC], BF16)
    m2_sb = mlp.tile([P, HT, TC], BF16)
    w1p, h_w1p = pool(name="wfc1", bufs=1, side="left")
    wfc1 = w1p.tile([P, ET, H], BF16)
    nc.sync.dma_start(wfc1[:], d["d_wfc1T"].rearrange("(m p) o -> p m o", p=P))
    w3p, h_w3p = pool(name="wfc3", bufs=1, side="left")
    wfc3 = w3p.tile([P, HT, E], BF16)
    nc.sync.dma_start(wfc3[:], d["d_wfc3T"].rearrange("(m p) o -> p m o", p=P))
    w2p, h_w2p = pool(name="wfc2c", bufs=1, side="left")
    wcs = []
    for e in range(HT):
        wc = w2p.tile([P, H], BF16, tag=f"wc{e}", name="wc")
        nc.sync.dma_start(wc[:], d["d_wfc2T"][e * P:(e + 1) * P, :])
        wcs.append(wc)
    warmup2(24, wfc1[:, 0, 0:256])           # bridge LN2 chain -> fc1
    for m in range(HT):
        ps1 = ps_mm.tile([P, TC], F32, tag="mm", name="ps1")
        for e in range(ET):
            nc.tensor.matmul(ps1[:], wfc1[:, e, m * P:(m + 1) * P],
                             h2_sb[:, e, :], start=(e == 0), stop=(e == ET - 1))
        nc.scalar.activation(m1_sb[:, m, :], ps1[:], AF.Relu,
                             bias=bfc1_sb[:, m:m + 1])
    close(h_ps_mm, STAT[0][1])

    # fc2: all 16 weight chunks resident -> one dense 256-matmul run
    ps8p, h_ps8p = pool(name="ps8", bufs=6, space="PSUM")
    for m in range(HT):
        psm = ps8p.tile([P, TC], F32, tag="mm8", name="psm")
        for e in range(HT):
            nc.tensor.matmul(psm[:], wcs[e][:, m * P:(m + 1) * P],
                             m1_sb[:, e, :],
                             start=(e == 0), stop=(e == HT - 1),
                             skip_group_check=True)
        nc.vector.tensor_scalar(m2_sb[:, m, :], psm[:],
                                scalar1=bfc2_sb[:, m:m + 1], scalar2=0.0,
                                op0=OP.add, op1=OP.max)
    close(h_ps8p, h_w2p)

    outp, h_outp = pool(name="outp", bufs=2, side="left")
    ps_f3, h_ps_f3 = pool(name="ps_f3", bufs=2, space="PSUM")
    for m in range(ET):
        ps3 = ps_f3.tile([P, TC], F32, tag="f3", name="ps3")
        for e in range(HT):
            nc.tensor.matmul(ps3[:], wfc3[:, e, m * P:(m + 1) * P],
                             m2_sb[:, e, :], start=(e == 0), stop=(e == HT - 1))
        ot = outp.tile([P, TC], F32, tag="ot", name="ot")
        nc.vector.scalar_tensor_tensor(
            ot[:], ps3[:], bfc3_sb[:, m:m + 1], x1_sb[:, m, :],
            op0=OP.add, op1=OP.add)
        nc.sync.dma_start(d["d_outT"][m * P:(m + 1) * P, :], ot[:])
    close(h_ps_f3, h_outp, h_w3p, h_w1p, h_mlp, h_persistB, h_persistA)
    close(_cd)
    close(_c3, _c2, _c1, _c0)


def _host_prep(inputs):
    """Host-side weight permutations / fp8 casts. Returns the common input map."""
    import ml_dtypes
    f8 = ml_dtypes.float8_e4m3
    f32 = np.float32

    qkv_w = np.asarray(inputs["qkv_w"], f32)
    qkv_b = np.asarray(inputs["qkv_b"], f32)
    wq_t = qkv_w[0:E].T.copy()          # [in E, out E]
    wk_t = qkv_w[E:2 * E].T.copy()
    wv_t = qkv_w[2 * E:3 * E].T.copy()
    bq, bk, bv = qkv_b[0:E], qkv_b[E:2 * E], qkv_b[2 * E:3 * E]

    # head-dim pair permutation: m-tile m, col pi = 64j + 32a + p holds
    # feature f = 64*(2m + a) + 2p + j  (head 2m+a, head-dim d = 2p+j)
    pi = np.arange(P)
    colf = np.empty((ET, P), np.int64)
    for m in range(ET):
        colf[m] = (64 * (2 * m + (pi // 32) % 2) + 2 * (pi % 32)
                   + pi // 64)
    colperm = colf.reshape(-1)

    def pair(wt):  # [K, out] -> [K//256, 128, 2, out] DoubleRow stationary
        K = wt.shape[0]
        return np.ascontiguousarray(
            wt.reshape(K // 256, 2, P, wt.shape[1]).transpose(0, 2, 1, 3))

    wk_p = wk_t[:, colperm]
    wq_p = wq_t[:, colperm]
    wkv8 = pair(np.concatenate([wk_p, wv_t], axis=1)).astype(f8)
    wq8 = pair(wq_p).astype(f8)

    proj_w = np.asarray(inputs["proj_w"], f32)
    bproj_eff = np.asarray(inputs["proj_b"], f32) + proj_w @ bv

    x = np.asarray(inputs["x"], f32)[0]          # [T, E]
    ct = lambda a: np.ascontiguousarray(np.asarray(a, f32).T)

    common = {
        "xT": ct(x).astype(ml_dtypes.bfloat16),
        "wkv8": wkv8,
        "wq8": wq8,
        "bqp": np.ascontiguousarray(bq[colperm].reshape(ET, P).T),
        "bkp": np.ascontiguousarray(bk[colperm].reshape(ET, P).T),
        "wproj8": pair(ct(proj_w)).astype(f8),
        "bproj": bproj_eff,
        "wfc1T": ct(inputs["fc1_w"]).astype(ml_dtypes.bfloat16),
        "bfc1": np.asarray(inputs["fc1_b"], f32),
        "wfc2T": ct(inputs["fc2_w"]).astype(ml_dtypes.bfloat16),
        "bfc2": np.asarray(inputs["fc2_b"], f32),
        "wfc3T": ct(inputs["fc3_w"]).astype(ml_dtypes.bfloat16),
        "bfc3": np.asarray(inputs["fc3_b"], f32),
        "lng": np.asarray(inputs["ln_g"], f32),
        "lnb": np.asarray(inputs["ln_b"], f32),
    }
    return common, x, ct


def _get_nc():
    global _BUILT
    if _BUILT is None:
        _BUILT = _build()
    return _BUILT


def run(inputs, trace=False):
    from concourse.bass_utils import run_bass_kernel_spmd

    nc = _get_nc()
    common, x, ct = _host_prep(inputs)
    in_maps = [
        {**common, "xsT": ct(x[c * TC:(c + 1) * TC, :])} for c in range(NCORES)
    ]
    res = run_bass_kernel_spmd(nc, in_maps, core_ids=list(range(NCORES)),
                               trace=trace)
    out = np.empty((1, T, E), np.float32)
    for c in range(NCORES):
        out[0, c * TC:(c + 1) * TC, :] = res.results[c]["outT"].T
    return out, res


def kernel(**inputs) -> np.ndarray:
    out, _ = run(inputs, trace=False)
    return out


# revision 38
# speedup vs baseline: 1.1681x; 1.0202x over previous
"""Trainium2 Bass kernel for a pre-norm transformer block (E=512, H=2048, NH=8, N=4096).

Sharding: sequence-parallel over 8 NeuronCores. Each core computes the full K/V
projection (needs all 4096 tokens) but only its own 512-token slice of queries,
attention output, MLP and residuals. No collectives; host concatenates slices.

v2: every large matmul runs in fp8(e4m3) DoubleRow perf mode (2 contraction
rows per PE cell, 0.5 cycles/row = 4x fewer PE cycles than bf16 at these
shapes). Contraction pairs are realized with host-side weight reshapes
([K, out] -> [K/256, 128, 2, out]); Q/K additionally get a head-dim pair
permutation (d -> (d//2, d%2)) so the 64-wide scores contraction becomes a
[32, 2, *] DoubleRow operand. Softmax exp is split across two engines by
query column: ACT runs native Exp -> fp8e5m2, DVE runs the Schraudolph exp2
bit trick (uint8 = round(0.72135*score + 60) bitcast to e5m2). Each softmax
row (head, q) is served by exactly one engine, so each row's uniform rounding
bias cancels in the softmax normalization (the denominator rides the PV
matmul as a 65th ones-column of V). The V bias is folded into the proj bias
on the host (exact algebra). The residual stream stays fp32.
"""
import sys

sys.path.insert(0, "/opt/trn_rl_repo")
sys.path.insert(0, "/opt/pypackages")

import numpy as np

E, H, NH, HD = 512, 2048, 8, 64
T, NCORES = 4096, 8
TC = T // NCORES          # tokens per core
P = 128
ET = E // P               # 4  feature tiles of E
HT = H // P               # 16 feature tiles of H
KT = T // P               # 32 key-token tiles
EPS = 1e-5
QA = 264                  # q columns per exp tile handled by ACT (rest: DVE)
EXP_A = 0.125 * 4 * np.log2(np.e)   # DVE bit-trick slope (e5m2, scores pre-scaled)
EXP_B = 60.0                        # e5m2 exponent-bias offset (4*15)

_BUILT = None


def _build():
    import concourse.bacc as bacc
    import concourse.mybir as mybir
    import concourse.tile as tile

    dt = mybir.dt
    F32 = dt.float32
    BF16 = dt.bfloat16
    F8 = dt.float8e4

    nc = bacc.Bacc("TRN2", target_bir_lowering=False, debug=False, num_devices=NCORES)

    d = {}
    d["d_xT"] = nc.dram_tensor("xT", [E, T], BF16, kind="ExternalInput").ap()
    d["d_xsT"] = nc.dram_tensor("xsT", [E, TC], F32, kind="ExternalInput").ap()
    # DoubleRow stationary layouts: [n_pair_groups, 128, 2, out_cols]
    d["d_wkv"] = nc.dram_tensor("wkv8", [2, P, 2, 2 * E], F8, kind="ExternalInput").ap()
    d["d_wq"] = nc.dram_tensor("wq8", [2, P, 2, E], F8, kind="ExternalInput").ap()
    d["d_wproj"] = nc.dram_tensor("wproj8", [2, P, 2, E], F8, kind="ExternalInput").ap()
    d["d_wfc1T"] = nc.dram_tensor("wfc1T", [E, H], BF16, kind="ExternalInput").ap()
    d["d_wfc2T"] = nc.dram_tensor("wfc2T", [H, H], BF16, kind="ExternalInput").ap()
    d["d_wfc3T"] = nc.dram_tensor("wfc3T", [H, E], BF16, kind="ExternalInput").ap()
    d["d_bqp"] = nc.dram_tensor("bqp", [P, ET], F32, kind="ExternalInput").ap()
    d["d_bkp"] = nc.dram_tensor("bkp", [P, ET], F32, kind="ExternalInput").ap()
    d["d_bproj"] = nc.dram_tensor("bproj", [E], F32, kind="ExternalInput").ap()
    d["d_bfc1"] = nc.dram_tensor("bfc1", [H], F32, kind="ExternalInput").ap()
    d["d_bfc2"] = nc.dram_tensor("bfc2", [H], F32, kind="ExternalInput").ap()
    d["d_bfc3"] = nc.dram_tensor("bfc3", [E], F32, kind="ExternalInput").ap()
    d["d_lng"] = nc.dram_tensor("lng", [E], F32, kind="ExternalInput").ap()
    d["d_lnb"] = nc.dram_tensor("lnb", [E], F32, kind="ExternalInput").ap()
    d["d_outT"] = nc.dram_tensor("outT", [E, TC], F32, kind="ExternalOutput").ap()

    with tile.TileContext(nc) as tc:
        _emit(nc, tc, tile, mybir, d)

    nc.compile()
    return nc


def _emit(nc, tc, tile, mybir, d):
    dt = mybir.dt
    AF = mybir.ActivationFunctionType
    OP = mybir.AluOpType
    PM = mybir.MatmulPerfMode
    F32 = dt.float32
    BF16 = dt.bfloat16
    F8 = dt.float8e4
    F8E5 = dt.float8e5
    U8 = dt.uint8

    def pool(**kw):
        p = tc.tile_pool(**kw)
        return p.__enter__(), p

    def close(*ps):
        for p in ps:
            p.__exit__(None, None, None)

    # ---- long-lived pools ----
    consts, _c0 = pool(name="consts", bufs=1, side="left")
    stats, _c1 = pool(name="stats", bufs=3, side="left")
    bcast, _c2 = pool(name="bcast", bufs=2, side="left")
    scratch, _c3 = pool(name="scratch", bufs=4, side="left")
    STAT = [pool(name="ps_stat", bufs=2, space="PSUM")]
    drp, _cd = pool(name="drscratch", bufs=4, space="DRAM")

    # ---- constants ----
    onesf = consts.tile([P, 1], F32)
    nc.vector.memset(onesf[:], 1.0)
    ones_w = consts.tile([P, 1], BF16)
    nc.vector.tensor_copy(ones_w[:], onesf[:])
    eps_t = consts.tile([1, 1], F32)
    nc.vector.memset(eps_t[:], EPS)
    eps_p = consts.tile([P, 1], F32)
    nc.vector.memset(eps_p[:], EPS)

    def ld_vec(dram, n, name):  # [n] f32 -> [P, n//P] per-partition layout
        t = consts.tile([P, n // P], F32, name=name)
        nc.sync.dma_start(t[:], dram.rearrange("(m p) -> p m", p=P))
        return t

    g_sb = ld_vec(d["d_lng"], E, "g_sb")
    b_sb = ld_vec(d["d_lnb"], E, "b_sb")
    bq_sb = consts.tile([P, ET], F32, name="bq_sb")
    nc.sync.dma_start(bq_sb[:], d["d_bqp"])
    bk_sb = consts.tile([P, ET], F32, name="bk_sb")
    nc.sync.dma_start(bk_sb[:], d["d_bkp"])
    bproj_sb = ld_vec(d["d_bproj"], E, "bproj_sb")
    bfc1_sb = ld_vec(d["d_bfc1"], H, "bfc1_sb")
    bfc2_sb = ld_vec(d["d_bfc2"], H, "bfc2_sb")
    bfc3_sb = ld_vec(d["d_bfc3"], E, "bfc3_sb")

    def bcast_recip(dst_bb, src_1w, w):
        """dst_bb[P, w] f32 = broadcast(1/src[1, w]) via DRAM stride-0 read +
        post-broadcast approx reciprocal (all partitions redundantly)."""
        dr = drp.tile([w], F32, tag="drb", name="drb")
        nc.sync.dma_start(dr[None, :], src_1w)
        nc.gpsimd.dma_start(dst_bb, dr[None, :].to_broadcast((P, w)))
        nc.vector.reciprocal_approx_fast(out=dst_bb, in_=dst_bb)

    def ln_stats(src, w, fast=False):
        """LN statistics from bf16 src tiles [P, ET, w]: (mu_b, rs_b) [P, w] bf16.

        Stats accumulate on the PE; the [1, w] -> [P, w] spread uses gpsimd
        partition_broadcast (no DRAM bounce), and rstd is computed redundantly
        on all partitions after the broadcast.
        """
        x2s = [scratch.tile([P, w], BF16, tag="ln_x2", name="ln_x2")
               for _ in range(ET)]
        for e in range(ET):
            nc.vector.tensor_mul(x2s[e][:], src[:, e, :], src[:, e, :])
        mu_ps = STAT[0][0].tile([1, w], F32, tag="mu")
        sq_ps = STAT[0][0].tile([1, w], F32, tag="sq")
        for e in range(ET):
            nc.tensor.matmul(mu_ps[:], ones_w[:], src[:, e, :],
                             start=(e == 0), stop=(e == ET - 1))
        for e in range(ET):
            nc.tensor.matmul(sq_ps[:], ones_w[:], x2s[e][:],
                             start=(e == 0), stop=(e == ET - 1))
        return ln_chain(mu_ps, sq_ps, w, fast=fast)

    def ln_chain(mu_ps, sq_ps, w, fast=False):
        # mean kept f32 on the fast path (x1 means are larger; bf16 there
        # costs visible accuracy); eps folds into the E[x^2] scale op
        mdt = F32 if fast else BF16
        mu_w = stats.tile([1, w], mdt, tag="mu_w", name="mu_w")
        sq_w = stats.tile([1, w], F32, tag="sq_w", name="sq_w")
        nc.vector.tensor_scalar_mul(mu_w[:], mu_ps[:], 1.0 / E)
        nc.scalar.activation(sq_w[:], sq_ps[:], AF.Identity, bias=eps_t[:],
                             scale=1.0 / E)
        mu_b = bcast.tile([P, w], mdt, tag="mu_b", name="mu_b")
        sq_b = bcast.tile([P, w], F32, tag="sq_b", name="sq_b")
        nc.gpsimd.partition_broadcast(mu_b[:], mu_w[:])
        nc.gpsimd.partition_broadcast(sq_b[:], sq_w[:])
        mu2 = scratch.tile([P, w], F32, tag="mu2", name="mu2")
        nc.vector.tensor_mul(mu2[:], mu_b[:], mu_b[:])
        nc.vector.tensor_sub(mu2[:], sq_b[:], mu2[:])
        nc.vector.reciprocal_approx_fast(out=mu2[:], in_=mu2[:])
        rs_b = bcast.tile([P, w], BF16, tag="rs_b", name="rs_b")
        nc.scalar.activation(rs_b[:], mu2[:], AF.Sqrt)
        return mu_b, rs_b

    def ln_apply(dst8, src, mu_b, rs_b, w, conv_act=False):
        """dst8[P, ET, w] = LN(src bf16); last op converts (one engine/tile)."""
        for e in range(ET):
            t1 = scratch.tile([P, w], BF16, tag="ln_t1", name="ln_t1")
            nc.vector.tensor_sub(t1[:], src[:, e, :], mu_b[:])
            nc.vector.scalar_tensor_tensor(t1[:], t1[:],
                                           g_sb[:, e:e + 1], rs_b[:],
                                           op0=OP.mult, op1=OP.mult)
            if conv_act:
                nc.scalar.activation(dst8[:, e, :], t1[:], AF.Identity,
                                     bias=b_sb[:, e:e + 1])
            else:
                nc.vector.tensor_scalar_add(dst8[:, e, :], t1[:],
                                            scalar1=b_sb[:, e:e + 1])

    # ---- K/V/Q tensors live until end of attention ----
    # K2/Q2 pair layout for DoubleRow scores: head h = 2u+a lives at
    # partitions [32a, 32a+32) of group u; (p, j) -> head dim d = 2p+j.
    big, h_big = pool(name="big", bufs=1, side="right")
    K2 = big.tile([HD, ET, 2, T], F8)           # [part (2 heads), u, j, tok]
    # V + ones column, padded to 80 so dual-fp8 LDWEIGHTS strides are
    # 16B-aligned (per-head 80, per-k-tile 640)
    V65 = big.tile([P, KT, NH, 80], F8)
    Q2 = big.tile([HD, ET, 2, TC], F8)

    # ones column of V65 (denominator rides the PV matmul)
    nc.vector.tensor_copy(
        V65[:, :, :, HD:HD + 1],
        onesf[:, :, None, None].to_broadcast((P, KT, NH, 1)))

    # ====== phase 1: stream x in 512-token chunks, LN, K/V proj ======
    ps_mm4, h_ps_mm4 = pool(name="ps_mm4", bufs=4, space="PSUM")
    CW = 512
    NCH1 = T // CW
    wqp, h_wqp = pool(name="wq", bufs=1, side="right")
    wq = wqp.tile([P, 2, 2, E], F8)             # [part, u_c, j_c, cols]
    wkvp, h_wkvp = pool(name="wkv", bufs=1, side="right")
    wkv = wkvp.tile([P, 2, 2, 2 * E], F8)       # cols: K-perm 512 | V 512
    xcp, h_xcp = pool(name="xc", bufs=3, side="right")
    hcp, h_hcp = pool(name="hc", bufs=3, side="right")

    def warmup(n, rhs, pool_=None):
        wps = (pool_ or ps_mm4).tile([1, rhs.shape[-1]], F32, tag="mm", name="wps")
        for i in range(n):
            nc.tensor.matmul(wps[:], ones_w[:], rhs,
                             start=(i == 0), stop=(i == n - 1),
                             skip_group_check=True)

    kqst, h_kqst = pool(name="kqst", bufs=6, side="right")

    def shuffle_pairs(dst, stg_t, m, cols):
        """Partition-shuffle staging [128, w] fp8 into the pair layout.

        Staging partition pi = 64j + 32a + p holds head (2m+a), d = 2p+j;
        dest is dst[0:64, m, j, cols] (one [64, w] DMA per j, queues split
        between sync and gpsimd)."""
        for j in range(2):
            q = nc.sync if j == 0 else nc.gpsimd
            q.dma_start(dst[0:64, m, j, cols],
                        stg_t[64 * j:64 * j + 64, :])

    def kv_project(hc, ch):
        # K: 4 dense m-tiles; stationary = host-permuted wkv K cols
        for m in range(ET):
            kps = ps_mm4.tile([P, CW], F32, tag="mm", name="kps")
            for u in range(2):
                nc.tensor.matmul(kps[:], wkv[:, u, :, m * P:(m + 1) * P],
                                 hc[:, 2 * u:2 * u + 2, :],
                                 start=(u == 0), stop=(u == 1),
                                 perf_mode=PM.DoubleRow)
            kst = kqst.tile([P, CW], F8, tag="kst", name="kst")
            nc.scalar.activation(kst[:], kps[:], AF.Identity,
                                 bias=bk_sb[:, m:m + 1])
            shuffle_pairs(K2, kst, m, slice(ch * CW, (ch + 1) * CW))
        # V: token-stationary, moving = wkv V cols -> [128 tok, 512 E]
        for t4 in range(CW // P):
            vps = ps_mm4.tile([P, E], F32, tag="mm", name="vps")
            for u in range(2):
                nc.tensor.matmul(vps[:], hc[:, 2 * u:2 * u + 2, t4 * P:(t4 + 1) * P],
                                 wkv[:, u, :, E:2 * E],
                                 start=(u == 0), stop=(u == 1),
                                 perf_mode=PM.DoubleRow)
            kt = ch * (CW // P) + t4
            if ch % 2 == 0:
                nc.scalar.activation(
                    V65[:, kt, :, 0:HD],
                    vps[:].rearrange("p (h d) -> p h d", h=NH), AF.Copy)
            else:
                nc.vector.tensor_copy(
                    V65[:, kt, :, 0:HD],
                    vps[:].rearrange("p (h d) -> p h d", h=NH))

    # Q projection resources (emitted mid-pipeline so it overlaps the K/V pass)
    persistA, h_persistA = pool(name="persistA", bufs=1, side="left")
    xs_sb = persistA.tile([P, ET, TC], F32)
    UTs = persistA.tile([P, ET, TC], F8)        # attention out (fp8, feature-major)
    persistQ, h_persistQ = pool(name="persistQ", bufs=1, side="left")
    xs_bf = persistQ.tile([P, ET, TC], BF16)
    hq = persistQ.tile([P, ET, TC], F8)

    def emit_phase2():
        nc.sync.dma_start(xs_sb[:], d["d_xsT"].rearrange("(m p) t -> p m t", p=P))
        for e in range(ET):
            nc.vector.tensor_copy(xs_bf[:, e, :], xs_sb[:, e, :])
        mu_b, rs_b = ln_chain_src(xs_bf, fast=True)
        ln_apply(hq, xs_bf, mu_b, rs_b, TC)
        for m in range(ET):
            qps = ps_mm4.tile([P, TC], F32, tag="mm", name="qps")
            for u in range(2):
                nc.tensor.matmul(qps[:], wq[:, u, :, m * P:(m + 1) * P],
                                 hq[:, 2 * u:2 * u + 2, :],
                                 start=(u == 0), stop=(u == 1),
                                 perf_mode=PM.DoubleRow)
            qst = kqst.tile([P, TC], F8, tag="kst", name="qst")
            nc.scalar.activation(qst[:], qps[:], AF.Identity,
                                 bias=bq_sb[:, m:m + 1])
            shuffle_pairs(Q2, qst, m, slice(0, TC))

    def ln_chain_src(src_bf, fast):
        return ln_stats(src_bf, TC, fast=fast)

    pending = []
    for ch in range(NCH1):
        xc = xcp.tile([P, ET, CW], BF16, tag="xc", name="xc")
        nc.sync.dma_start(
            xc[:],
            d["d_xT"][:, ch * CW:(ch + 1) * CW].rearrange("(m p) t -> p m t", p=P))
        if ch == 0:
            warmup(32, xc[:, 0, 0:256])      # bridge the LN latency at t=0
            nc.sync.dma_start(wkv[:], d["d_wkv"].rearrange("u p j o -> p u j o"))
            nc.sync.dma_start(wq[:], d["d_wq"].rearrange("u p j o -> p u j o"))
        mu_b, rs_b = ln_stats(xc, CW)
        hc = hcp.tile([P, ET, CW], F8, tag="hc", name="hc")
        ln_apply(hc, xc, mu_b, rs_b, CW, conv_act=(ch % 2 == 1))
        pending.append((hc, ch))
        if len(pending) == 2:
            kv_project(*pending.pop(0))
    while pending:
        kv_project(*pending.pop(0))
    emit_phase2()
    warmup(32, wkv[:, 0, 0, 0:256])          # bridge Q-chain -> attention start
    close(h_kqst, h_hcp, h_xcp, h_wkvp, h_wqp)
    close(h_ps_mm4)
    close(STAT[0][1])
    close(h_persistQ)

    # ====== phase 3: attention (2 heads in flight, DoubleRow everywhere) ======
    # Software-pipelined: PV(ktp) is emitted AFTER scores(ktp+1) so the
    # in-order PE queue never head-of-line blocks on the exp chain.
    ps_sc, h_ps_sc = pool(name="ps_sc", bufs=3, space="PSUM")
    ps_pv, h_ps_pv = pool(name="ps_pv", bufs=1, space="PSUM")
    ptp, h_ptp = pool(name="ptile", bufs=6, side="right")
    stp, h_stp = pool(name="stage", bufs=2, side="right")
    for mp in range(ET):
        heads = [2 * mp, 2 * mp + 1]
        pvs = [ps_pv.tile([HD + 1, TC], F32, tag=f"pv{j}", name="pv")
               for j in range(2)]

        def emit_scores(ktp):
            k0 = 2 * ktp
            pts = []
            for j, h in enumerate(heads):
                u_o, a = h // 2, h % 2
                lo = 32 * a
                sc2 = ps_sc.tile([P, 2 * TC], F32, tag="sc2", name="sc2")
                for kk in range(2):
                    nc.tensor.matmul(
                        sc2[:, kk * TC:(kk + 1) * TC],
                        K2[lo:lo + 32, u_o, :, (k0 + kk) * P:(k0 + kk + 1) * P],
                        Q2[lo:lo + 32, u_o, :, :],
                        perf_mode=PM.DoubleRow, skip_group_check=True)
                scv = sc2[:].rearrange("p (kk q) -> p kk q", kk=2)
                # exp split into single-producer tiles: ACT native Exp on
                # q cols [0, QA), DVE bit-trick on [QA, TC)
                ptua = ptp.tile([P, 2, QA], U8, tag="pta", name="pta")
                ptub = ptp.tile([P, 2, TC - QA], U8, tag="ptb", name="ptb")
                pt8a = ptua[:].bitcast(F8E5)
                nc.scalar.activation(pt8a, scv[:, :, 0:QA], AF.Exp, scale=0.125)
                nc.vector.tensor_scalar(ptub[:], scv[:, :, QA:TC],
                                        scalar1=float(EXP_A), scalar2=float(EXP_B),
                                        op0=OP.mult, op1=OP.add)
                pts.append((pt8a, ptub[:].bitcast(F8E5)))
            return pts

        def emit_pv(ktp, pts):
            # PSUM start zeroing is 2KB-region granular: only the first
            # sub-matmul may carry start=True (its mark covers the whole
            # bank; the b-columns zero-fill on their own first write).
            k0 = 2 * ktp
            for j, h in enumerate(heads):
                nc.tensor.matmul(pvs[j][:, 0:QA], V65[:, k0:k0 + 2, h, 0:HD + 1],
                                 pts[j][0],
                                 start=(ktp == 0), stop=(ktp == KT // 2 - 1),
                                 perf_mode=PM.DoubleRow, skip_group_check=True)
                nc.tensor.matmul(pvs[j][:, QA:TC], V65[:, k0:k0 + 2, h, 0:HD + 1],
                                 pts[j][1],
                                 start=False, stop=(ktp == KT // 2 - 1),
                                 perf_mode=PM.DoubleRow, skip_group_check=True)

        prev = None
        for ktp in range(KT // 2):
            pts = emit_scores(ktp)
            if prev is not None:
                emit_pv(ktp - 1, prev)
            prev = pts
        emit_pv(KT // 2 - 1, prev)
        for j, h in enumerate(heads):
            lo = 64 * (h % 2)
            m = h // 2
            pv = pvs[j]
            den = stp.tile([HD + 1, TC], F32, tag="den", name="den")
            nc.scalar.activation(den[HD:HD + 1, :], pv[HD:HD + 1, :], AF.Copy)
            rb = bcast.tile([P, TC], F32, tag="rb", name="rb")
            bcast_recip(rb[:], den[HD:HD + 1, :], TC)
            stg = stp.tile([HD, TC], F8, tag="stg", name="stg")
            nc.vector.tensor_mul(stg[:], pv[0:HD, :], rb[0:HD, :])
            nc.sync.dma_start(UTs[lo:lo + HD, m, :], stg[:])
    close(h_stp, h_ptp, h_ps_pv, h_ps_sc)
    close(h_big)                     # K/V/Q dead after attention

    # ============ phase 4: output proj + residual + LN2 ============
    STAT[0] = pool(name="ps_stat2", bufs=2, space="PSUM")
    ps_mm, h_ps_mm = pool(name="ps_mm", bufs=4, space="PSUM")
    persistB, h_persistB = pool(name="persistB", bufs=1, side="left")
    x1_sb = persistB.tile([P, ET, TC], F32)
    x1_bf = persistB.tile([P, ET, TC], BF16)
    h2_sb = persistB.tile([P, ET, TC], BF16)
    wpp, h_wpp = pool(name="wproj", bufs=1, side="left")
    wproj = wpp.tile([P, 2, 2, E], F8)
    nc.sync.dma_start(wproj[:], d["d_wproj"].rearrange("u p j o -> p u j o"))

    def warmup2(n, rhs):
        wps = ps_mm.tile([1, rhs.shape[-1]], F32, tag="mm", name="wps")
        for i in range(n):
            nc.tensor.matmul(wps[:], ones_w[:], rhs,
                             start=(i == 0), stop=(i == n - 1),
                             skip_group_check=True)

    warmup2(32, wproj[:, 0, 0, 0:256])       # bridge attention tail -> proj
    mu2_ps = STAT[0][0].tile([1, TC], F32, tag="mu")
    sq2_ps = STAT[0][0].tile([1, TC], F32, tag="sq")
    for m in range(ET):
        pps = ps_mm.tile([P, TC], F32, tag="mm", name="pps")
        for u in range(2):
            nc.tensor.matmul(pps[:], wproj[:, u, :, m * P:(m + 1) * P],
                             UTs[:, 2 * u:2 * u + 2, :],
                             start=(u == 0), stop=(u == 1),
                             perf_mode=PM.DoubleRow)
        # x1 = (proj + bias) + x_slice
        nc.vector.scalar_tensor_tensor(
            x1_sb[:, m, :], pps[:], bproj_sb[:, m:m + 1], xs_sb[:, m, :],
            op0=OP.add, op1=OP.add)
        # LN2 statistics accumulate as each x1 block lands
        nc.scalar.activation(x1_bf[:, m, :], x1_sb[:, m, :], AF.Copy)
        x2 = scratch.tile([P, TC], BF16, tag="ln_x2", name="ln_x2")
        nc.vector.tensor_mul(x2[:], x1_bf[:, m, :], x1_bf[:, m, :])
        nc.tensor.matmul(mu2_ps[:], ones_w[:], x1_bf[:, m, :],
                         start=(m == 0), stop=(m == ET - 1), skip_group_check=True)
        nc.tensor.matmul(sq2_ps[:], ones_w[:], x2[:],
                         start=(m == 0), stop=(m == ET - 1), skip_group_check=True)
    close(h_wpp)
    mu_b2, rs_b2 = ln_chain(mu2_ps, sq2_ps, TC, fast=True)
    ln_apply(h2_sb, x1_bf, mu_b2, rs_b2, TC)

    # ============ phase 5: MLP (bf16 — fp8 is too lossy for the MLP) ============
    mlp, h_mlp = pool(name="mlp", bufs=1, side="left")
    m1_sb = mlp.tile([P, HT, TC], BF16)
    m2_sb = mlp.tile([P, HT, TC], BF16)
    w1p, h_w1p = pool(name="wfc1", bufs=1, side="left")
    wfc1 = w1p.tile([P, ET, H], BF16)
    nc.sync.dma_start(wfc1[:], d["d_wfc1T"].rearrange("(m p) o -> p m o", p=P))
    w3p, h_w3p = pool(name="wfc3", bufs=1, side="left")
    wfc3 = w3p.tile([P, HT, E], BF16)
    nc.sync.dma_start(wfc3[:], d["d_wfc3T"].rearrange("(m p) o -> p m o", p=P))
    w2p, h_w2p = pool(name="wfc2c", bufs=1, side="left")
    wcs = []
    for e in range(HT):
        wc = w2p.tile([P, H], BF16, tag=f"wc{e}", name="wc")
        nc.sync.dma_start(wc[:], d["d_wfc2T"][e * P:(e + 1) * P, :])
        wcs.append(wc)
    warmup2(24, wfc1[:, 0, 0:256])           # bridge LN2 chain -> fc1
    for m in range(HT):
        ps1 = ps_mm.tile([P, TC], F32, tag="mm", name="ps1")
        for e in range(ET):
            nc.tensor.matmul(ps1[:], wfc1[:, e, m * P:(m + 1) * P],
                             h2_sb[:, e, :], start=(e == 0), stop=(e == ET - 1))
        nc.scalar.activation(m1_sb[:, m, :], ps1[:], AF.Relu,
                             bias=bfc1_sb[:, m:m + 1])
    close(h_ps_mm, STAT[0][1])

    # fc2: all 16 weight chunks resident -> one dense 256-matmul run
    ps8p, h_ps8p = pool(name="ps8", bufs=6, space="PSUM")
    for m in range(HT):
        psm = ps8p.tile([P, TC], F32, tag="mm8", name="psm")
        for e in range(HT):
            nc.tensor.matmul(psm[:], wcs[e][:, m * P:(m + 1) * P],
                             m1_sb[:, e, :],
                             start=(e == 0), stop=(e == HT - 1),
                             skip_group_check=True)
        nc.vector.tensor_scalar(m2_sb[:, m, :], psm[:],
                                scalar1=bfc2_sb[:, m:m + 1], scalar2=0.0,
                                op0=OP.add, op1=OP.max)
    close(h_ps8p, h_w2p)

    outp, h_outp = pool(name="outp", bufs=2, side="left")
    ps_f3, h_ps_f3 = pool(name="ps_f3", bufs=2, space="PSUM")
    for m in range(ET):
        ps3 = ps_f3.tile([P, TC], F32, tag="f3", name="ps3")
        for e in range(HT):
            nc.tensor.matmul(ps3[:], wfc3[:, e, m * P:(m + 1) * P],
                             m2_sb[:, e, :], start=(e == 0), stop=(e == HT - 1))
        ot = outp.tile([P, TC], F32, tag="ot", name="ot")
        nc.vector.scalar_tensor_tensor(
            ot[:], ps3[:], bfc3_sb[:, m:m + 1], x1_sb[:, m, :],
            op0=OP.add, op1=OP.add)
        nc.sync.dma_start(d["d_outT"][m * P:(m + 1) * P, :], ot[:])
    close(h_ps_f3, h_outp, h_w3p, h_w1p, h_mlp, h_persistB, h_persistA)
    close(_cd)
    close(_c3, _c2, _c1, _c0)


def _host_prep(inputs):
    """Host-side weight permutations / fp8 casts. Returns the common input map."""
    import ml_dtypes
    f8 = ml_dtypes.float8_e4m3
    f32 = np.float32

    qkv_w = np.asarray(inputs["qkv_w"], f32)
    qkv_b = np.asarray(inputs["qkv_b"], f32)
    wq_t = qkv_w[0:E].T.copy()          # [in E, out E]
    wk_t = qkv_w[E:2 * E].T.copy()
    wv_t = qkv_w[2 * E:3 * E].T.copy()
    bq, bk, bv = qkv_b[0:E], qkv_b[E:2 * E], qkv_b[2 * E:3 * E]

    # head-dim pair permutation: m-tile m, col pi = 64j + 32a + p holds
    # feature f = 64*(2m + a) + 2p + j  (head 2m+a, head-dim d = 2p+j)
    pi = np.arange(P)
    colf = np.empty((ET, P), np.int64)
    for m in range(ET):
        colf[m] = (64 * (2 * m + (pi // 32) % 2) + 2 * (pi % 32)
                   + pi // 64)
    colperm = colf.reshape(-1)

    def pair(wt):  # [K, out] -> [K//256, 128, 2, out] DoubleRow stationary
        K = wt.shape[0]
        return np.ascontiguousarray(
            wt.reshape(K // 256, 2, P, wt.shape[1]).transpose(0, 2, 1, 3))

    wk_p = wk_t[:, colperm]
    wq_p = wq_t[:, colperm]
    wkv8 = pair(np.concatenate([wk_p, wv_t], axis=1)).astype(f8)
    wq8 = pair(wq_p).astype(f8)

    proj_w = np.asarray(inputs["proj_w"], f32)
    bproj_eff = np.asarray(inputs["proj_b"], f32) + proj_w @ bv

    x = np.asarray(inputs["x"], f32)[0]          # [T, E]
    ct = lambda a: np.ascontiguousarray(np.asarray(a, f32).T)

    common = {
        "xT": ct(x).astype(ml_dtypes.bfloat16),
        "wkv8": wkv8,
        "wq8": wq8,
        "bqp": np.ascontiguousarray(bq[colperm].reshape(ET, P).T),
        "bkp": np.ascontiguousarray(bk[colperm].reshape(ET, P).T),
        "wproj8": pair(ct(proj_w)).astype(f8),
        "bproj": bproj_eff,
        "wfc1T": ct(inputs["fc1_w"]).astype(ml_dtypes.bfloat16),
        "bfc1": np.asarray(inputs["fc1_b"], f32),
        "wfc2T": ct(inputs["fc2_w"]).astype(ml_dtypes.bfloat16),
        "bfc2": np.asarray(inputs["fc2_b"], f32),
        "wfc3T": ct(inputs["fc3_w"]).astype(ml_dtypes.bfloat16),
        "bfc3": np.asarray(inputs["fc3_b"], f32),
        "lng": np.asarray(inputs["ln_g"], f32),
        "lnb": np.asarray(inputs["ln_b"], f32),
    }
    return common, x, ct


def _get_nc():
    global _BUILT
    if _BUILT is None:
        _BUILT = _build()
    return _BUILT


def run(inputs, trace=False):
    from concourse.bass_utils import run_bass_kernel_spmd

    nc = _get_nc()
    common, x, ct = _host_prep(inputs)
    in_maps = [
        {**common, "xsT": ct(x[c * TC:(c + 1) * TC, :])} for c in range(NCORES)
    ]
    res = run_bass_kernel_spmd(nc, in_maps, core_ids=list(range(NCORES)),
                               trace=trace)
    out = np.empty((1, T, E), np.float32)
    for c in range(NCORES):
        out[0, c * TC:(c + 1) * TC, :] = res.results[c]["outT"].T
    return out, res


def kernel(**inputs) -> np.ndarray:
    out, _ = run(inputs, trace=False)
    return out
